# revision 1
# baseline (speedup 1.0000x reference)
"""Trainium2 Bass kernel for nn_DiscretePredictor (gnn_message_passing).

Reference computation (per batch b of 256, n=32 objects, d=128):
    src/tgt = all n*n ordered pairs (i,j), e = edges[b, i*n+j] in {0,1}
    need[(b,i,j)] = [state[b,i]*e, state[b,j]*e]              (2d = 256)
    msgs = MLP1(need) = Lin(256->256) -> BN(train) -> LeakyReLU -> Lin(256->128)
    agg[b,i] = sum_j msgs[b,i,j]
    out = MLP2([state, agg]) same structure (256->256->BN->LReLU->128)

Key algebraic facts exploited:
  1. need @ W1.T = e*(u_i + v_j) with u = state@W1a.T, v = state@W1b.T
     (W1 = [W1a | W1b] split along input dim), so the 262144x256x256 matmul
     collapses to two 8192x128x256 matmuls plus an elementwise outer-add.
  2. sum_j commutes with the second linear layer:
     agg = (sum_j LeakyReLU(BN(h))) @ W2.T + n*b2.
  3. e in {0,1}  =>  LeakyReLU masking is exact: the masked pre-activation is
     H = e*(u_i+v_j); rows with e=0 reduce to the constant LeakyReLU(z0).
  4. Training-mode BN uses global batch stats => two tiny (128x4) AllReduduces
     across the 8 cores.  The BN linear bias b1 cancels:
     BN(h) = a*H + (beta - mean(H)*a), a = gamma*rsqrt(var(H)+eps).

Sharding: data-parallel over batch (32 batches per core), params replicated.

Per-core dataflow (feature-major: features on SBUF partitions).  The BN1
stats are computed WITHOUT materializing H, so the cross-core stats barrier
happens before the big elementwise phase and H never has to be spilled:
  stateT [128d, 1024(b,i)] --PE--> UT/VT [2][128f, 1024]
  sum(H)   = sum_i deg*u + sum_j cdeg*v              (tiny DVE mul+reduce)
  sum(H^2) = sum deg*u^2 + sum cdeg*v^2 + 2*w1a_f^T M w1b_f,
             M = sum_b S_b^T E_b S_b                 (small PE matmuls via
             block-diag E_b^T tiles loaded from a host-transposed edges copy)
  AllReduce [128,4] -> BN coeffs a1[f], z0[f]
  fold a1 into the weights: redo u/v matmuls with a1*W1a, a1*W1b -> UA/VA
  big phase, per group g (4 batches x 2 f-halves, 16 tiles of [128, 4096]):
    DVE  tensor_add : W = ua_i + va_j     (stride-0 broadcast APs)
    POOL tensor_mul : H = W * E_bcast     (e in {0,1}: exact masking)
    ACT  Prelu      : m = LeakyReLU(H + z0)  (z0 via per-partition bias)
    DVE  reduce_sum : msum[f,(b,i)] += sum_j m   (j innermost)
  PE: aggT = W2 @ msum + 32*b2 ; H2 = FW1 @ [stateT; aggT]
  ACT copy+accum / Square+accum -> layer-2 stats; AllReduce; Prelu -> m2
  PE: outT = FW2 @ m2 + fb2 ; DMA out (host transposes back)

Engine budget per core (cost model): DVE ~138us (add+reduce), POOL ~130us
(mask), ACT ~75us, PE ~35us; modelled end-to-end ~293us.
"""

import os
import sys

for p in ("/opt/trn_rl_repo", "/root/.axon_site", "/root/.axon_site/_ro/trn_rl_repo",
          "/root/.axon_site/_ro/pypackages"):
    if os.path.isdir(p) and p not in sys.path:
        sys.path.append(p)

import numpy as np

import concourse.bass as bass
import concourse.mybir as mybir
import concourse.tile as tile
from concourse import bacc
from concourse.bass_utils import run_bass_kernel_spmd

F32 = mybir.dt.float32
AF = mybir.ActivationFunctionType
ALU = mybir.AluOpType

B = 256          # global batch
NOBJ = 32        # objects per batch
D = 128          # object dim
F = 256          # hidden width (both MLPs)
NCORES = 8
NB = B // NCORES          # batches per core = 32
ROWS = NB * NOBJ          # (b,i) rows per core = 1024
GB = 4                    # batches per stage-B group
NG = NB // GB             # 8 groups
CG = GB * NOBJ * NOBJ     # stage-B cols per group = 4096
N1 = float(B * NOBJ * NOBJ)   # BN1 row count (global) = 262144
N2 = float(B * NOBJ)          # BN2 row count (global) = 8192
EPS = 1e-5
SLOPE = 0.01
# debug: skip cross-core allreduce (stats become shard-local; wrong numerics)
NO_CC = os.environ.get("BASS_NO_CC", "0") == "1"
STAGE = int(os.environ.get("BASS_STAGE", "9"))  # debug: emit pipeline prefix only
SUB = int(os.environ.get("BASS_SUB", "9"))      # debug: const-load subset


def _build_nc():
    nc = bacc.Bacc("TRN2", target_bir_lowering=False, debug=False,
                   enable_asserts=True, num_devices=NCORES)

    # ---- per-core device I/O ----
    stateT_d = nc.dram_tensor("stateT", [D, ROWS], F32, kind="ExternalInput")
    edges_d = nc.dram_tensor("edges_s", [NB, NOBJ * NOBJ], F32, kind="ExternalInput")
    w1aT_d = nc.dram_tensor("w1aT", [D, F], F32, kind="ExternalInput")
    w1bT_d = nc.dram_tensor("w1bT", [D, F], F32, kind="ExternalInput")
    w2T_d = nc.dram_tensor("w2T", [F, D], F32, kind="ExternalInput")
    fw1T_d = nc.dram_tensor("fw1T", [2 * D, F], F32, kind="ExternalInput")
    fw2T_d = nc.dram_tensor("fw2T", [F, D], F32, kind="ExternalInput")
    g1_d = nc.dram_tensor("g1", [F], F32, kind="ExternalInput")
    be1_d = nc.dram_tensor("be1", [F], F32, kind="ExternalInput")
    b2_d = nc.dram_tensor("b2", [D], F32, kind="ExternalInput")
    g2_d = nc.dram_tensor("g2", [F], F32, kind="ExternalInput")
    be2_d = nc.dram_tensor("be2", [F], F32, kind="ExternalInput")
    fb2_d = nc.dram_tensor("fb2", [D], F32, kind="ExternalInput")
    staterm_d = nc.dram_tensor("state_rm", [ROWS, D], F32, kind="ExternalInput")
    edgesT_d = nc.dram_tensor("edgesT_s", [NB, NOBJ * NOBJ], F32, kind="ExternalInput")
    outT_d = nc.dram_tensor("outT", [D, ROWS], F32, kind="ExternalOutput")

    from contextlib import ExitStack
    with tile.TileContext(nc) as tc, ExitStack() as ctx:
        consts = ctx.enter_context(tc.tile_pool(name="consts", bufs=1))
        uvp = ctx.enter_context(tc.tile_pool(name="uv", bufs=1))
        big = ctx.enter_context(tc.tile_pool(name="big", bufs=2))
        statp = ctx.enter_context(tc.tile_pool(name="stats", bufs=1))
        psum = ctx.enter_context(tc.tile_pool(name="psum", bufs=4, space="PSUM"))
        dram = ctx.enter_context(tc.tile_pool(name="dram", bufs=1, space="DRAM"))

        # ---------------- setup: load params + state ----------------
        sT = consts.tile([D, ROWS], F32)
        nc.sync.dma_start(out=sT[:], in_=stateT_d.ap())
        w1a = consts.tile([D, F], F32)
        w1b = consts.tile([D, F], F32)
        w2k = consts.tile([D, 2, D], F32)  # [k-half][128,128] tiles of w2T
        fw1 = consts.tile([D, 2, F], F32)  # [128, k-half, 256]
        fw2 = consts.tile([D, 2, D], F32)
        nc.sync.dma_start(out=w1a[:], in_=w1aT_d.ap())
        nc.sync.dma_start(out=w1b[:], in_=w1bT_d.ap())
        if SUB >= 2:
            nc.sync.dma_start(out=w2k[:], in_=w2T_d.ap().rearrange("(k p) d -> p k d", p=D))
            nc.sync.dma_start(out=fw1[:], in_=fw1T_d.ap().rearrange("(k p) f -> p k f", p=D))
            nc.sync.dma_start(out=fw2[:], in_=fw2T_d.ap().rearrange("(k p) d -> p k d", p=D))

        def fvec(dh, nm):  # [256] dram vector -> [128, 2] feature-major sbuf
            t = consts.tile([D, 2], F32, tag=nm, name=nm)
            nc.sync.dma_start(out=t[:], in_=dh.ap().rearrange("(h p) -> p h", p=D))
            return t

        def dvec(dh, nm):  # [128] -> [128, 1]
            t = consts.tile([D, 1], F32, tag=nm, name=nm)
            nc.sync.dma_start(out=t[:], in_=dh.ap().rearrange("(h p) -> p h", p=D))
            return t

        g1c, be1c = fvec(g1_d, "g1c"), fvec(be1_d, "be1c")
        g2c, be2c = fvec(g2_d, "g2c"), fvec(be2_d, "be2c")
        b2c, fb2c = dvec(b2_d, "b2c"), dvec(fb2_d, "fb2c")
        b2x32 = consts.tile([D, 1], F32)
        nc.vector.tensor_scalar_mul(b2x32[:], b2c[:], float(NOBJ))

        if STAGE <= 0:
            nc.sync.dma_start(out=outT_d.ap(), in_=sT[:])
            return nc
        # ---------------- u/v matmuls:  UT/VT[fh] = [128f, 1024(b,*)] --------
        UT = [uvp.tile([D, ROWS], F32, tag=f"UT{h}", name=f"UT{h}") for h in range(2)]
        VT = [uvp.tile([D, ROWS], F32, tag=f"VT{h}", name=f"VT{h}") for h in range(2)]
        for fh in range(2):
            for dst, w in ((UT, w1a), (VT, w1b)):
                for nh in range(2):
                    ps = psum.tile([D, 512], F32, bufs=3)
                    nc.tensor.matmul(ps[:], w[:, fh * D:(fh + 1) * D],
                                     sT[:, nh * 512:(nh + 1) * 512],
                                     start=True, stop=True)
                    nc.scalar.activation(out=dst[fh][:, nh * 512:(nh + 1) * 512],
                                         in_=ps[:], func=AF.Copy)

        # ---------------- sum(H) via degrees:  sum e*(u+v) = deg.u + cdeg.v ---
        esb = consts.tile([NB, NOBJ * NOBJ], F32)
        nc.sync.dma_start(out=esb[:], in_=edges_d.ap())
        deg = statp.tile([NB, NOBJ], F32)    # [b, i] row degree
        nc.vector.reduce_sum(deg[:], esb[:].rearrange("p (i j) -> p i j", j=NOBJ),
                             axis=mybir.AxisListType.X)
        cdeg = statp.tile([NB, NOBJ], F32)   # [b, j] col degree
        nc.vector.reduce_sum(cdeg[:], esb[:].rearrange("p (i j) -> p j i", j=NOBJ),
                             axis=mybir.AxisListType.X)
        degd = dram.tile([NB, NOBJ], F32, tag="degd")
        cdegd = dram.tile([NB, NOBJ], F32, tag="cdegd")
        nc.sync.dma_start(out=degd[:], in_=deg[:])
        nc.sync.dma_start(out=cdegd[:], in_=cdeg[:])
        degrep = statp.tile([D, ROWS], F32)
        nc.sync.dma_start(
            out=degrep[:].rearrange("p (b i) -> p b i", i=NOBJ),
            in_=degd[:].partition_broadcast(D))
        cdegrep = statp.tile([D, ROWS], F32)
        nc.sync.dma_start(
            out=cdegrep[:].rearrange("p (b j) -> p b j", j=NOBJ),
            in_=cdegd[:].partition_broadcast(D))
        shpart = statp.tile([D, 4], F32)     # col = fh*2 + {u,v}
        ttrs = statp.tile([D, ROWS], F32)
        for fh in range(2):
            for uv, (src, rep) in enumerate(((UT[fh], degrep), (VT[fh], cdegrep))):
                nc.vector.tensor_mul(ttrs[:], src[:], rep[:])
                nc.vector.reduce_sum(shpart[:, 2 * fh + uv:2 * fh + uv + 1], ttrs[:],
                                     axis=mybir.AxisListType.X)

        # ---------------- sum(H^2) via algebra ------------------------------
        # sum e*(u+v)^2 = sum_i deg*u^2 + sum_j cdeg*v^2 + 2*sum e*u*v
        # cross term: sum_f' e u v = w1a_f^T M w1b_f,  M = sum_b S_b^T E_b S_b
        sq4 = statp.tile([D, 4], F32)        # col = fh*2 + {u2, v2}
        for fh in range(2):
            for uv, (src_, rep) in enumerate(((UT[fh], degrep), (VT[fh], cdegrep))):
                usq = big.tile([D, ROWS], F32, tag="w", bufs=3, name=f"usq{fh}{uv}")
                nc.scalar.activation(out=usq[:], in_=src_[:], func=AF.Square)
                nc.vector.tensor_mul(ttrs[:], usq[:], rep[:])
                nc.vector.reduce_sum(sq4[:, 2 * fh + uv:2 * fh + uv + 1], ttrs[:],
                                     axis=mybir.AxisListType.X)

        ones_mat = statp.tile([D, D], F32)
        nc.vector.memset(ones_mat[:], 1.0)
        mps = psum.tile([D, D], F32, tag="mps", bufs=1, name="mps")
        for g in range(NG):
            spack = big.tile([D, D], F32, tag="spack", bufs=3, name=f"spack{g}")
            nc.sync.dma_start(out=spack[:], in_=staterm_d.ap()[g * D:(g + 1) * D, :])
            etb = big.tile([D, D], F32, tag="etb", bufs=3, name=f"etb{g}")
            nc.vector.memset(etb[:], 0.0)
            for bs in range(GB):
                nc.sync.dma_start(
                    out=etb[bs * NOBJ:(bs + 1) * NOBJ, bs * NOBJ:(bs + 1) * NOBJ],
                    in_=edgesT_d.ap()[g * GB + bs].rearrange("(j i) -> j i", i=NOBJ))
            esg = psum.tile([D, D], F32, tag="esg", bufs=2, name=f"esg{g}")
            nc.tensor.matmul(esg[:], etb[:], spack[:], start=True, stop=True)
            esgs = big.tile([D, D], F32, tag="esgs", bufs=3, name=f"esgs{g}")
            nc.scalar.activation(out=esgs[:], in_=esg[:], func=AF.Copy)
            nc.tensor.matmul(mps[:], spack[:], esgs[:],
                             start=(g == 0), stop=(g == NG - 1))
        msb = statp.tile([D, D], F32)
        nc.scalar.activation(out=msb[:], in_=mps[:], func=AF.Copy)
        t1p = psum.tile([D, F], F32, tag="t1p", bufs=1, name="t1p")
        nc.tensor.matmul(t1p[:], msb[:], w1a[:], start=True, stop=True)
        t2 = statp.tile([D, F], F32)
        nc.vector.tensor_mul(t2[:], t1p[:], w1b[:])
        crp = psum.tile([D, F], F32, tag="crp", bufs=1, name="crp")
        nc.tensor.matmul(crp[:], ones_mat[:], t2[:], start=True, stop=True)
        crsb = statp.tile([1, F], F32)
        nc.scalar.activation(out=crsb[:], in_=crp[0:1, :], func=AF.Copy)
        crd = dram.tile([1, F], F32, tag="crd", name="crd")
        nc.sync.dma_start(out=crd[:], in_=crsb[:])
        crossc = statp.tile([D, 2], F32)
        nc.sync.dma_start(out=crossc[:],
                          in_=crd[:].rearrange("x (h p) -> (x p) h", p=D))

        # ---------------- BN1 stats allreduce + coeffs ----------------
        stat1 = statp.tile([D, 4], F32)
        nc.vector.reduce_sum(stat1[:, 0:2],
                             shpart[:].rearrange("p (fh uv) -> p fh uv", uv=2),
                             axis=mybir.AxisListType.X)
        sqsum = statp.tile([D, 2], F32)
        nc.vector.reduce_sum(sqsum[:],
                             sq4[:].rearrange("p (fh uv) -> p fh uv", uv=2),
                             axis=mybir.AxisListType.X)
        cr2 = statp.tile([D, 2], F32)
        nc.vector.tensor_scalar_mul(cr2[:], crossc[:], 2.0)
        nc.vector.tensor_add(stat1[:, 2:4], sqsum[:], cr2[:])
        cc1_in = dram.tile([D, 4], F32, tag="cc1i")
        cc1_out = dram.tile([D, 4], F32, tag="cc1o")
        nc.sync.dma_start(out=cc1_in[:], in_=stat1[:])
        if NO_CC:
            nc.sync.dma_start(out=cc1_out[:], in_=cc1_in[:])
        else:
            nc.gpsimd.collective_compute(
                "AllReduce", ALU.add, replica_groups=[list(range(NCORES))],
                ins=[cc1_in[:].opt()], outs=[cc1_out[:].opt()])
        statg1 = statp.tile([D, 4], F32)
        nc.sync.dma_start(out=statg1[:], in_=cc1_out[:])

        if STAGE <= 2:
            nc.sync.dma_start(out=outT_d.ap(), in_=degrep[:])
            return nc
        epsc = statp.tile([D, 1], F32)
        nc.vector.memset(epsc[:], EPS)

        def bn_coeffs(statg, n_rows, gc, bec, pool, pre):
            # statg [128, 4] = [sumH(2fh), sumH2(2fh)] -> a=[128,2], z=[128,2]
            def tl(nm):
                return pool.tile([D, 2], F32, tag=pre + nm, name=pre + nm)
            mean, msq, var = tl("mean"), tl("msq"), tl("var")
            sd, rstd, a, ma, z = tl("sd"), tl("rstd"), tl("a"), tl("ma"), tl("z")
            nc.scalar.activation(out=mean[:], in_=statg[:, 0:2], func=AF.Copy,
                                 scale=1.0 / n_rows)
            nc.scalar.activation(out=msq[:], in_=mean[:], func=AF.Square)
            # var = statg[2:4]/N - mean^2   (one fused scalar_tensor_tensor)
            nc.vector.scalar_tensor_tensor(
                out=var[:], in0=statg[:, 2:4], scalar=1.0 / n_rows,
                in1=msq[:], op0=ALU.mult, op1=ALU.subtract)
            nc.scalar.activation(out=sd[:], in_=var[:], func=AF.Sqrt, bias=epsc[:])
            nc.vector.reciprocal(out=rstd[:], in_=sd[:])
            nc.vector.tensor_mul(a[:], gc[:], rstd[:])
            # z = beta - mean*a
            nc.vector.tensor_mul(ma[:], mean[:], a[:])
            nc.vector.tensor_sub(z[:], bec[:], ma[:])
            return a, z

        a1, z1 = bn_coeffs(statg1, N1 / (NCORES if NO_CC else 1), g1c, be1c, statp, "bn1_")

        # ---------------- fold a1 into u/v: redo matmuls with a1*W1 ----------
        a1d = dram.tile([1, F], F32, tag="a1d", name="a1d")
        nc.sync.dma_start(out=a1d[:].rearrange("x (h p) -> (x p) h", p=D), in_=a1[:])
        a1rep = statp.tile([D, F], F32)
        nc.sync.dma_start(out=a1rep[:],
                          in_=a1d[:].rearrange("x f -> (x f)").partition_broadcast(D))
        w1as = statp.tile([D, F], F32)
        nc.vector.tensor_mul(w1as[:], w1a[:], a1rep[:])
        w1bs = statp.tile([D, F], F32)
        nc.vector.tensor_mul(w1bs[:], w1b[:], a1rep[:])
        UA = [uvp.tile([D, ROWS], F32, tag=f"UT{h}", name=f"UA{h}") for h in range(2)]
        VA = [uvp.tile([D, ROWS], F32, tag=f"VT{h}", name=f"VA{h}") for h in range(2)]
        for fh in range(2):
            for dst, w in ((UA, w1as), (VA, w1bs)):
                for nh in range(2):
                    ps = psum.tile([D, 512], F32, bufs=3)
                    nc.tensor.matmul(ps[:], w[:, fh * D:(fh + 1) * D],
                                     sT[:, nh * 512:(nh + 1) * 512],
                                     start=True, stop=True)
                    nc.scalar.activation(out=dst[fh][:, nh * 512:(nh + 1) * 512],
                                         in_=ps[:], func=AF.Copy)

        # ------- big phase: W = ua+va ; H = W*e ; m = LRelu(H + z0) ; sum_j --
        msumS = [statp.tile([D, ROWS], F32, tag=f"msum{h}", name=f"msum{h}") for h in range(2)]
        pend = None  # (mt, fh, g) whose j-reduce is deferred one tile (DVE FIFO)
        for g in range(NG):
            erep = big.tile([D, CG], F32, tag="erep", bufs=2)
            esl = edges_d.ap()[g * GB:(g + 1) * GB, :]
            nc.gpsimd.dma_start(
                out=erep[:].rearrange("p (b c) -> p b c", b=GB),
                in_=esl.partition_broadcast(D))
            for fh in range(2):
                csl = slice(g * GB * NOBJ, (g + 1) * GB * NOBJ)
                u4 = (UA[fh][:, csl].rearrange("p (b i) -> p b i", i=NOBJ)
                      .unsqueeze(3).broadcast_to([D, GB, NOBJ, NOBJ]))
                v4 = (VA[fh][:, csl].rearrange("p (b j) -> p b j", j=NOBJ)
                      .unsqueeze(2).broadcast_to([D, GB, NOBJ, NOBJ]))
                wt = big.tile([D, CG], F32, tag="w", bufs=3, name=f"wt{g}{fh}")
                nc.vector.tensor_add(
                    wt[:].rearrange("p (b i j) -> p b i j", i=NOBJ, j=NOBJ), u4, v4)
                ht = big.tile([D, CG], F32, tag="h", bufs=3, name=f"ht{g}{fh}")
                nc.gpsimd.tensor_mul(
                    ht[:].rearrange("p (b c) -> p b c", b=GB),
                    wt[:].rearrange("p (b c) -> p b c", b=GB),
                    erep[:].rearrange("p (b c) -> p b c", b=GB))
                mt = big.tile([D, CG], F32, tag="w", bufs=3, name=f"mt{g}{fh}")
                nc.scalar.activation(out=mt[:], in_=ht[:], func=AF.Prelu,
                                     scale=1.0, bias=z1[:, fh:fh + 1], alpha=SLOPE)
                if pend is not None:
                    pmt, pfh, pg = pend
                    nc.vector.reduce_sum(
                        out=msumS[pfh][:, pg * GB * NOBJ:(pg + 1) * GB * NOBJ],
                        in_=pmt[:].rearrange("p (r j) -> p r j", j=NOBJ),
                        axis=mybir.AxisListType.X)
                pend = (mt, fh, g)
        pmt, pfh, pg = pend
        nc.vector.reduce_sum(
            out=msumS[pfh][:, pg * GB * NOBJ:(pg + 1) * GB * NOBJ],
            in_=pmt[:].rearrange("p (r j) -> p r j", j=NOBJ),
            axis=mybir.AxisListType.X)

        if STAGE <= 4:
            nc.sync.dma_start(out=outT_d.ap(), in_=msumS[0][:])
            return nc
        # ---------------- aggT = 32*(W2 @ avg + b2) ----------------
        aggT = big.tile([D, ROWS], F32, tag="h", bufs=3, name="aggT")
        for nh in range(2):
            ps = psum.tile([D, 512], F32, bufs=3)
            nc.tensor.matmul(ps[:], w2k[:, 0, :], msumS[0][:, nh * 512:(nh + 1) * 512],
                             start=True, stop=False)
            nc.tensor.matmul(ps[:], w2k[:, 1, :], msumS[1][:, nh * 512:(nh + 1) * 512],
                             start=False, stop=True)
            nc.scalar.activation(out=aggT[:, nh * 512:(nh + 1) * 512], in_=ps[:],
                                 func=AF.Identity, bias=b2x32[:], scale=1.0)

        # ---------------- layer 2: H2 = FW1 @ [sT; aggT], stats -------------
        H2 = [statp.tile([D, ROWS], F32, tag=f"h2_{h}", name=f"h2_{h}") for h in range(2)]
        st2part = statp.tile([D, 8], F32)  # col = s*4 + fh*2 + nh
        for fh in range(2):
            for nh in range(2):
                ps = psum.tile([D, 512], F32, bufs=3)
                nc.tensor.matmul(ps[:], fw1[:, 0, fh * D:(fh + 1) * D],
                                 sT[:, nh * 512:(nh + 1) * 512], start=True, stop=False)
                nc.tensor.matmul(ps[:], fw1[:, 1, fh * D:(fh + 1) * D],
                                 aggT[:, nh * 512:(nh + 1) * 512], start=False, stop=True)
                c1 = 0 * 4 + fh * 2 + nh
                c2 = 1 * 4 + fh * 2 + nh
                nc.scalar.activation(out=H2[fh][:, nh * 512:(nh + 1) * 512], in_=ps[:],
                                     func=AF.Copy,
                                     accum_out=st2part[:, c1:c1 + 1])
                sq2 = statp.tile([D, 512], F32, tag="sq2")
                nc.scalar.activation(out=sq2[:], in_=H2[fh][:, nh * 512:(nh + 1) * 512],
                                     func=AF.Square,
                                     accum_out=st2part[:, c2:c2 + 1])

        stat2 = statp.tile([D, 4], F32)  # [sumH2(2fh), sumH2sq(2fh)]
        nc.vector.reduce_sum(stat2[:],
                             st2part[:].rearrange("p (s fh nh) -> p (s fh) nh", s=2, nh=2),
                             axis=mybir.AxisListType.X)
        cc2_in = dram.tile([D, 4], F32, tag="cc2i")
        cc2_out = dram.tile([D, 4], F32, tag="cc2o")
        nc.sync.dma_start(out=cc2_in[:], in_=stat2[:])
        if NO_CC:
            nc.sync.dma_start(out=cc2_out[:], in_=cc2_in[:])
        else:
            nc.gpsimd.collective_compute(
                "AllReduce", ALU.add, replica_groups=[list(range(NCORES))],
                ins=[cc2_in[:].opt()], outs=[cc2_out[:].opt()])
        statg2 = statp.tile([D, 4], F32)
        nc.sync.dma_start(out=statg2[:], in_=cc2_out[:])
        a2, z2 = bn_coeffs(statg2, N2 / (NCORES if NO_CC else 1), g2c, be2c, statp, "bn2_")

        # ---------------- m2 = Prelu(a2*H2+z2); outT = FW2 @ m2 + fb2 -------
        m2 = [big.tile([D, ROWS], F32, tag="w", bufs=3, name=f"m2_{h}") for h in range(2)]
        for fh in range(2):
            nc.scalar.activation(out=m2[fh][:], in_=H2[fh][:], func=AF.Prelu,
                                 scale=a2[:, fh:fh + 1], bias=z2[:, fh:fh + 1],
                                 alpha=SLOPE)
        outT = big.tile([D, ROWS], F32, tag="h", bufs=3, name="outT")
        for nh in range(2):
            ps = psum.tile([D, 512], F32, bufs=3)
            nc.tensor.matmul(ps[:], fw2[:, 0, :], m2[0][:, nh * 512:(nh + 1) * 512],
                             start=True, stop=False)
            nc.tensor.matmul(ps[:], fw2[:, 1, :], m2[1][:, nh * 512:(nh + 1) * 512],
                             start=False, stop=True)
            nc.scalar.activation(out=outT[:, nh * 512:(nh + 1) * 512], in_=ps[:],
                                 func=AF.Identity, bias=fb2c[:], scale=1.0)
        nc.sync.dma_start(out=outT_d.ap(), in_=outT[:])
    return nc


def _build_nc_staged():
    nc = _build_nc()
    nc.compile()
    return nc


_NC_CACHE = {}


def _get_nc():
    if "nc" not in _NC_CACHE:
        _NC_CACHE["nc"] = _build_nc_staged()
    return _NC_CACHE["nc"]


def _make_in_maps(inputs):
    return _prep_in_maps(**inputs)


def _prep_in_maps(state, edges, msg_w1, msg_b1, msg_gamma, msg_beta, msg_w2,
                  msg_b2, fin_w1, fin_b1, fin_gamma, fin_beta, fin_w2, fin_b2,
                  **_unused):
    f32 = np.float32
    state = np.ascontiguousarray(np.asarray(state, f32))
    edges = np.ascontiguousarray(np.asarray(edges, f32))

    # replicated params, pre-transposed to device layout (lhsT = K x M)
    w1aT = np.ascontiguousarray(np.asarray(msg_w1, f32)[:, :D].T)    # [128, 256]
    w1bT = np.ascontiguousarray(np.asarray(msg_w1, f32)[:, D:].T)    # [128, 256]
    w2T = np.ascontiguousarray(np.asarray(msg_w2, f32).T)            # [256, 128]
    fw1T = np.ascontiguousarray(np.asarray(fin_w1, f32).T)           # [256, 256]
    fw2T = np.ascontiguousarray(np.asarray(fin_w2, f32).T)           # [256, 128]
    shared = {
        "w1aT": w1aT, "w1bT": w1bT, "w2T": w2T, "fw1T": fw1T, "fw2T": fw2T,
        "g1": np.ascontiguousarray(np.asarray(msg_gamma, f32)),
        "be1": np.ascontiguousarray(np.asarray(msg_beta, f32)),
        "b2": np.ascontiguousarray(np.asarray(msg_b2, f32)),
        "g2": np.ascontiguousarray(np.asarray(fin_gamma, f32)),
        "be2": np.ascontiguousarray(np.asarray(fin_beta, f32)),
        "fb2": np.ascontiguousarray(np.asarray(fin_b2, f32)),
    }
    in_maps = []
    for c in range(NCORES):
        sh = state[c * NB:(c + 1) * NB].reshape(ROWS, D)
        ed = edges[c * NB:(c + 1) * NB]
        in_maps.append({
            "stateT": np.ascontiguousarray(sh.T),
            "state_rm": np.ascontiguousarray(sh),
            "edges_s": np.ascontiguousarray(ed),
            "edgesT_s": np.ascontiguousarray(
                ed.reshape(NB, NOBJ, NOBJ).transpose(0, 2, 1).reshape(NB, -1)),
            **shared,
        })
    return in_maps


def kernel(**inputs):
    in_maps = _prep_in_maps(**inputs)
    nc = _get_nc()
    res = run_bass_kernel_spmd(nc, in_maps, core_ids=list(range(NCORES)))
    out = np.empty((B, NOBJ, D), np.float32)
    for c in range(NCORES):
        outT = res.results[c]["outT"]                       # [128, 1024]
        out[c * NB:(c + 1) * NB] = outT.T.reshape(NB, NOBJ, D)
    return out


if __name__ == "__main__":
    rng = np.random.default_rng(0)
    print("smoke-building nc...")
    _get_nc()
    print("built OK")



# revision 15
# speedup vs baseline: 1.6269x; 1.6269x over previous
"""Trainium2 Bass kernel for nn_DiscretePredictor (gnn_message_passing), v2.

Reference (per batch b of 256, n=32 objects, d=128):
    e = edges[b, i*n+j] in {0,1}
    msgs = MLP1([state_i*e, state_j*e]) : Lin(256->256) -> BN -> LReLU -> Lin(256->128)
    agg[b,i] = sum_j msgs ;  out = MLP2([state, agg]) same shape.

v2 strategy (vs v1 which did the masked pre-activation elementwise on DVE/Pool):
  * H = e*(u_i + v_j) is built ON THE PE as one K=64 matmul per (batch,
    f-half): lhsT = [u_b rows; v_b rows] (32+32 partitions), rhs = PQ_b, a
    host-precomputed block-diagonal operand with P[k,(i,j)] = e[i,j]*d(k==i)
    (routes+masks u) stacked on Q[k,(i,j)] = e[i,j]*d(k==j) (routes+masks v).
    This deletes the two 34-65us elementwise passes (add + mask) entirely.
  * All matmuls run in bf16 (1 cyc/row vs 4 for fp32).
  * BN1 scale/shift are applied inside the Prelu activation (scale=a1,
    bias=z1 per-partition APs) - no re-matmul with folded weights.
  * BN1 stats come from tiny PE matmuls: sum(H) = sum deg*u + cdeg*v and
    sum(H^2) = sum deg*u^2 + cdeg*v^2 + 2*w1a^T M w1b with M = sum S^T E S,
    using a [128,16] host-built deg/cdeg vector matched to the uv layout.
  * The j-reduction is a packed-bf16 halving tree on DVE (2x mode) for some
    units and a straight Pool tensor_reduce for the rest; a few Prelu units
    run on DVE (tensor_scalar fused a1*H+z1, then max(t, slope*t)) to
    balance the three engines.

Sharding: data-parallel over batch (32 batches/core), params replicated,
two [128,4] AllReduces for the sync-BN stats.
"""

import os
import sys

for p in ("/opt/trn_rl_repo", "/root/.axon_site", "/root/.axon_site/_ro/trn_rl_repo",
          "/root/.axon_site/_ro/pypackages"):
    if os.path.isdir(p) and p not in sys.path:
        sys.path.append(p)

import numpy as np
import ml_dtypes

import concourse.bass as bass
import concourse.mybir as mybir
import concourse.tile as tile
from concourse import bacc
from concourse.bass_utils import run_bass_kernel_spmd

F32 = mybir.dt.float32
BF16 = mybir.dt.bfloat16
AF = mybir.ActivationFunctionType
ALU = mybir.AluOpType
NPBF = ml_dtypes.bfloat16

B = 256          # global batch
NOBJ = 32        # objects per batch
D = 128          # object dim
F = 256          # hidden width (both MLPs)
NCORES = 8
NB = B // NCORES          # batches per core = 32
ROWS = NB * NOBJ          # (b,i) rows per core = 1024
NS = NB // 2              # pair-slots (2 batches each) = 16
N1 = float(B * NOBJ * NOBJ)   # BN1 row count (global)
N2 = float(B * NOBJ)          # BN2 row count (global)
EPS = 1e-5
SLOPE = 0.01
NO_CC = os.environ.get("BASS_NO_CC", "0") == "1"

# engine-balance knobs: units are (s, fh), 32 total
DVE_PRELU = int(os.environ.get("BASS_DVE_PRELU", "5"))    # units preluing on DVE
POOL_RED = int(os.environ.get("BASS_POOL_RED", "22"))     # units whose tree-L1 runs on Pool


def _unit_flags():
    """Spread DVE-prelu and Pool-reduce units evenly over the 32 units."""
    units = [(s, fh) for s in range(NS) for fh in range(2)]
    n = len(units)
    dve_p = set()
    if DVE_PRELU > 0:
        step = n / DVE_PRELU
        dve_p = {units[min(n - 1, int(i * step))] for i in range(DVE_PRELU)}
    pool_r = set()
    if POOL_RED > 0:
        step = n / POOL_RED
        pool_r = {units[min(n - 1, int(i * step + 0.5))] for i in range(POOL_RED)}
    return dve_p, pool_r


def _build_nc():
    nc = bacc.Bacc("TRN2", target_bir_lowering=False, debug=False,
                   enable_asserts=True, num_devices=NCORES)

    # ---- per-core device I/O ----
    sT_d = nc.dram_tensor("stateT", [D, ROWS], BF16, kind="ExternalInput")
    srm_d = nc.dram_tensor("state_rm", [ROWS, D], BF16, kind="ExternalInput")
    pq_d = nc.dram_tensor("pq", [D, NS * 2 * NOBJ * NOBJ], BF16, kind="ExternalInput")
    wdeg_d = nc.dram_tensor("wdeg", [D, NS], BF16, kind="ExternalInput")
    edgesT_d = nc.dram_tensor("edgesT_s", [NB, NOBJ * NOBJ], BF16, kind="ExternalInput")
    w1a_d = nc.dram_tensor("w1aT", [D, F], BF16, kind="ExternalInput")
    w1b_d = nc.dram_tensor("w1bT", [D, F], BF16, kind="ExternalInput")
    w2k_d = nc.dram_tensor("w2T", [F, D], F32, kind="ExternalInput")
    fw1_d = nc.dram_tensor("fw1T", [2 * D, F], BF16, kind="ExternalInput")
    fw2_d = nc.dram_tensor("fw2T", [F, D], BF16, kind="ExternalInput")
    g1_d = nc.dram_tensor("g1", [F], F32, kind="ExternalInput")
    be1_d = nc.dram_tensor("be1", [F], F32, kind="ExternalInput")
    b2_d = nc.dram_tensor("b2", [D], F32, kind="ExternalInput")
    g2_d = nc.dram_tensor("g2", [F], F32, kind="ExternalInput")
    be2_d = nc.dram_tensor("be2", [F], F32, kind="ExternalInput")
    fb2_d = nc.dram_tensor("fb2", [D], F32, kind="ExternalInput")
    outT_d = nc.dram_tensor("outT", [D, ROWS], F32, kind="ExternalOutput")

    dve_prelu_units, pool_red_units = _unit_flags()

    from contextlib import ExitStack
    with tile.TileContext(nc) as tc, ExitStack() as ctx:
        consts = ctx.enter_context(tc.tile_pool(name="consts", bufs=1))
        uvp = ctx.enter_context(tc.tile_pool(name="uv", bufs=1))
        big = ctx.enter_context(tc.tile_pool(name="big", bufs=2))
        statp = ctx.enter_context(tc.tile_pool(name="stats", bufs=1))
        dram = ctx.enter_context(tc.tile_pool(name="dram", bufs=1, space="DRAM"))

        # ---------------- setup: load params + state ----------------
        sT = consts.tile([D, ROWS], BF16)
        nc.sync.dma_start(out=sT[:], in_=sT_d.ap())
        pqt = consts.tile([D, NS, 2 * NOBJ * NOBJ], BF16)
        pq_src = pq_d.ap().rearrange("p (s c) -> p s c", s=NS)
        for q in range(4):
            nc.sync.dma_start(out=pqt[:, 4 * q:4 * (q + 1), :],
                              in_=pq_src[:, 4 * q:4 * (q + 1), :])
        wdeg = consts.tile([D, NS], BF16)
        nc.sync.dma_start(out=wdeg[:], in_=wdeg_d.ap())
        w1a = consts.tile([D, F], BF16)
        w1b = consts.tile([D, F], BF16)
        nc.sync.dma_start(out=w1a[:], in_=w1a_d.ap())
        nc.sync.dma_start(out=w1b[:], in_=w1b_d.ap())
        w2k = consts.tile([D, 2, D], F32)
        fw1 = consts.tile([D, 2, F], BF16)
        fw2 = consts.tile([D, 2, D], BF16)
        nc.sync.dma_start(out=w2k[:], in_=w2k_d.ap().rearrange("(k p) d -> p k d", p=D))
        nc.sync.dma_start(out=fw1[:], in_=fw1_d.ap().rearrange("(k p) f -> p k f", p=D))
        nc.sync.dma_start(out=fw2[:], in_=fw2_d.ap().rearrange("(k p) d -> p k d", p=D))

        def fvec(dh, nm):  # [256] dram vector -> [128, 2] feature-major sbuf
            t = consts.tile([D, 2], F32, tag=nm, name=nm)
            nc.sync.dma_start(out=t[:], in_=dh.ap().rearrange("(h p) -> p h", p=D))
            return t

        def dvec(dh, nm):  # [128] -> [128, 1]
            t = consts.tile([D, 1], F32, tag=nm, name=nm)
            nc.sync.dma_start(out=t[:], in_=dh.ap().rearrange("(h p) -> p h", p=D))
            return t

        g1c, be1c = fvec(g1_d, "g1c"), fvec(be1_d, "be1c")
        g2c, be2c = fvec(g2_d, "g2c"), fvec(be2_d, "be2c")
        b2c, fb2c = dvec(b2_d, "b2c"), dvec(fb2_d, "fb2c")
        b2x32 = consts.tile([D, 1], F32)
        nc.vector.tensor_scalar_mul(b2x32[:], b2c[:], float(NOBJ))
        ones1 = consts.tile([D, 1], BF16)
        nc.vector.memset(ones1[:], 1.0)

        # ------------- uvT matmuls: interleaved [u_b0; v_b0; u_b1; v_b1] ----
        # uvT[p, s, f]: p in [0,32) u rows of batch 2s (i=p), [32,64) v rows
        # of 2s, [64,96) u of 2s+1, [96,128) v of 2s+1.
        uvT = uvp.tile([D, NS, F], BF16)
        uvSq = uvp.tile([D, NS, F], BF16)
        sc1 = statp.tile([1, 2 * F], F32)
        psA_cm = tc.tile_pool(name="psA", bufs=2, space="PSUM")
        psA = psA_cm.__enter__()
        for s in range(NS):
            ps = psA.tile([D, F], F32, tag="uvps", bufs=2)
            rsl = slice(2 * s * NOBJ, (2 * s + 2) * NOBJ)   # 64 rows: b0|b1
            nc.tensor.matmul(ps[0:64, :], sT[:, rsl], w1a[:],
                             start=True, stop=True)
            nc.tensor.matmul(ps[64:128, :], sT[:, rsl], w1b[:],
                             start=True, stop=True)
            nc.gpsimd.tensor_scalar_mul(uvT[:, s, :], ps[:], 1.0)
        with nc.allow_low_precision(reason="bf16 squares feed fp32 psum sums"):
            for h in range(2):
                nc.vector.tensor_mul(uvSq[:, 8 * h:8 * (h + 1), :],
                                     uvT[:, 8 * h:8 * (h + 1), :],
                                     uvT[:, 8 * h:8 * (h + 1), :])

        # ------------- BN1 stats: sum(H), sum(H^2) ----------------
        # sum(H)[f]  = sum_s sum_p wdeg[p,s]*uvT[p,s,f]   (deg for u rows,
        # cdeg for v rows, both batches)  -> [1, 256] psum accumulate
        shp = psA.tile([1, F], F32, tag="shp", bufs=1, name="shp")
        sqp = psA.tile([1, F], F32, tag="sqp", bufs=1, name="sqp")
        for s in range(NS):
            nc.tensor.matmul(shp[:], wdeg[:, s:s + 1], uvT[:, s, :],
                             start=(s == 0), stop=(s == NS - 1))
        for s in range(NS):
            nc.tensor.matmul(sqp[:], wdeg[:, s:s + 1], uvSq[:, s, :],
                             start=(s == 0), stop=(s == NS - 1))

        # cross term: sum_f' e*u*v = w1a_f^T M w1b_f, M = sum_b S_b^T E_b S_b
        mps = psA.tile([D, D], F32, tag="mps", bufs=1, name="mps")
        for g in range(NB // 4):  # 8 groups of 4 batches = 128 rows
            spack = big.tile([D, D], BF16, tag="spack", bufs=2, name=f"spack{g}")
            nc.sync.dma_start(out=spack[:], in_=srm_d.ap()[g * D:(g + 1) * D, :])
            etb = big.tile([D, D], BF16, tag="etb", bufs=2, name=f"etb{g}")
            nc.vector.memset(etb[:], 0.0)
            for bs in range(4):
                nc.sync.dma_start(
                    out=etb[bs * NOBJ:(bs + 1) * NOBJ, bs * NOBJ:(bs + 1) * NOBJ],
                    in_=edgesT_d.ap()[g * 4 + bs].rearrange("(j i) -> j i", i=NOBJ))
            esg = psA.tile([D, D], F32, tag="esg", bufs=2, name=f"esg{g}")
            nc.tensor.matmul(esg[:], etb[:], spack[:], start=True, stop=True)
            esgs = big.tile([D, D], BF16, tag="esgs", bufs=2, name=f"esgs{g}")
            nc.gpsimd.tensor_scalar_mul(esgs[:], esg[:], 1.0)
            nc.tensor.matmul(mps[:], spack[:], esgs[:],
                             start=(g == 0), stop=(g == NB // 4 - 1))
        msb = statp.tile([D, D], BF16)
        nc.scalar.activation(out=msb[:], in_=mps[:], func=AF.Copy)
        t1p = psA.tile([D, F], F32, tag="uvps", name="t1p")
        nc.tensor.matmul(t1p[:], msb[:], w1a[:], start=True, stop=True)
        t2 = statp.tile([D, F], BF16)
        with nc.allow_low_precision(reason="cross-term partial"):
            nc.vector.tensor_mul(t2[:], t1p[:], w1b[:])
        crp = psA.tile([1, F], F32, tag="crp", bufs=1, name="crp")
        nc.tensor.matmul(crp[:], ones1[:], t2[:], start=True, stop=True)

        # pack [sumH(256) | sumH2(256)] on one partition, roundtrip via DRAM
        # to feature-major [128, 4], AllReduce, then BN coeffs.
        nc.scalar.activation(out=sc1[:, 0:F], in_=shp[:], func=AF.Copy)
        nc.vector.scalar_tensor_tensor(out=sc1[:, F:2 * F], in0=crp[:], scalar=2.0,
                                       in1=sqp[:], op0=ALU.mult, op1=ALU.add)
        st1d = dram.tile([1, 2 * F], F32, tag="st1d", name="st1d")
        nc.sync.dma_start(out=st1d[:], in_=sc1[:])
        cc1_in = dram.tile([D, 4], F32, tag="cc1i")
        nc.sync.dma_start(out=cc1_in[:],
                          in_=st1d[:].rearrange("x (g h p) -> (x p) (g h)", p=D, g=2))
        cc1_out = dram.tile([D, 4], F32, tag="cc1o")
        if NO_CC:
            nc.sync.dma_start(out=cc1_out[:], in_=cc1_in[:])
        else:
            nc.gpsimd.collective_compute(
                "AllReduce", ALU.add, replica_groups=[list(range(NCORES))],
                ins=[cc1_in[:].opt()], outs=[cc1_out[:].opt()])
        statg1 = statp.tile([D, 4], F32)
        nc.sync.dma_start(out=statg1[:], in_=cc1_out[:])

        epsc = statp.tile([D, 1], F32)
        nc.vector.memset(epsc[:], EPS)

        def bn_coeffs(statg, n_rows, gc, bec, pool, pre):
            def tl(nm):
                return pool.tile([D, 2], F32, tag=pre + nm, name=pre + nm)
            mean, msq, var = tl("mean"), tl("msq"), tl("var")
            sd, rstd, a, ma, z = tl("sd"), tl("rstd"), tl("a"), tl("ma"), tl("z")
            nc.scalar.activation(out=mean[:], in_=statg[:, 0:2], func=AF.Copy,
                                 scale=1.0 / n_rows)
            nc.scalar.activation(out=msq[:], in_=mean[:], func=AF.Square)
            nc.vector.scalar_tensor_tensor(
                out=var[:], in0=statg[:, 2:4], scalar=1.0 / n_rows,
                in1=msq[:], op0=ALU.mult, op1=ALU.subtract)
            nc.scalar.activation(out=sd[:], in_=var[:], func=AF.Sqrt, bias=epsc[:])
            nc.vector.reciprocal(out=rstd[:], in_=sd[:])
            nc.vector.tensor_mul(a[:], gc[:], rstd[:])
            nc.vector.tensor_mul(ma[:], mean[:], a[:])
            nc.vector.tensor_sub(z[:], bec[:], ma[:])
            return a, z

        a1, z1 = bn_coeffs(statg1, N1 / (NCORES if NO_CC else 1), g1c, be1c,
                           statp, "bn1_")

        # ------------- big phase: H on PE, Prelu, j-reduce ----------------
        # per unit (s, fh): H[128, 2048] = [b0 cols | b1 cols] in PSUM,
        #   m = Prelu(a1*H + z1)  (ACT, or DVE 2-instr for some units)
        #   msum[:, s*64:(s+1)*64] = sum_j m  (DVE halving tree or Pool)
        msumS = [statp.tile([D, ROWS], F32, tag=f"msum{h}", name=f"msum{h}")
                 for h in range(2)]
        psA_cm.__exit__(None, None, None)  # release psA banks for the big phase
        psB_cm = tc.tile_pool(name="psB", bufs=2, space="PSUM")
        psB = psB_cm.__enter__()
        CG2 = 2 * NOBJ * NOBJ  # 2048
        for s in range(NS):
            for fh in range(2):
                hps = psB.tile([D, CG2], F32, tag="hps", bufs=2)
                nc.tensor.matmul(hps[:], uvT[:, s, fh * D:(fh + 1) * D],
                                 pqt[:, s, :], start=True, stop=True)
                mt = big.tile([D, CG2], BF16, tag="mt", bufs=3, name=f"mt{s}{fh}")
                if (s, fh) in dve_prelu_units:
                    tf = big.tile([D, CG2], F32, tag="tf", bufs=2, name=f"tf{s}{fh}")
                    nc.vector.tensor_scalar(
                        out=tf[:], in0=hps[:], scalar1=a1[:, fh:fh + 1],
                        scalar2=z1[:, fh:fh + 1], op0=ALU.mult, op1=ALU.add)
                    with nc.allow_low_precision(reason="prelu out"):
                        nc.vector.scalar_tensor_tensor(
                            out=mt[:], in0=tf[:], scalar=SLOPE, in1=tf[:],
                            op0=ALU.mult, op1=ALU.max)
                else:
                    nc.scalar.activation(out=mt[:], in_=hps[:], func=AF.Prelu,
                                         scale=a1[:, fh:fh + 1],
                                         bias=z1[:, fh:fh + 1], alpha=SLOPE)
                osl = msumS[fh][:, s * 64:(s + 1) * 64]
                # packed-bf16 halving tree over j (2x DVE mode); level 1 may
                # run on Pool to balance engines
                cur = mt[:].rearrange("p (r j) -> p r j", j=NOBJ)
                w = NOBJ
                with nc.allow_low_precision(reason="bf16 j-tree"):
                    while w > 2:
                        w //= 2
                        nt = big.tile([D, 64 * w], BF16, tag=f"tr{w}",
                                      bufs=2, name=f"tr{s}{fh}{w}")
                        nv = nt[:].rearrange("p (r j) -> p r j", j=w)
                        eng = (nc.gpsimd if (w == NOBJ // 2 and
                                             (s, fh) in pool_red_units)
                               else nc.vector)
                        eng.tensor_add(nv, cur[:, :, 0:w], cur[:, :, w:2 * w])
                        cur = nv
                nc.vector.tensor_add(osl.rearrange("p (r x) -> p r x", x=1),
                                     cur[:, :, 0:1], cur[:, :, 1:2])

        # ------------- aggT = W2 @ msum + 32*b2 ; H2 = FW1 @ [sT; aggT] -----
        psB_cm.__exit__(None, None, None)
        psC = ctx.enter_context(tc.tile_pool(name="psC", bufs=2, space="PSUM"))
        aggT = statp.tile([D, ROWS], BF16, name="aggT")
        H2 = [statp.tile([D, ROWS], F32, tag=f"h2_{h}", name=f"h2_{h}")
              for h in range(2)]
        st2sum = statp.tile([D, 4], F32)   # col = fh*2 + nh : sum H2
        st2sq = statp.tile([D, 4], F32)    # col = fh*2 + nh : sum H2^2
        for nh in range(2):
            csl = slice(nh * 512, (nh + 1) * 512)
            ps = psC.tile([D, 512], F32, tag="aggp", bufs=2)
            nc.tensor.matmul(ps[:], w2k[:, 0, :], msumS[0][:, csl],
                             start=True, stop=False)
            nc.tensor.matmul(ps[:], w2k[:, 1, :], msumS[1][:, csl],
                             start=False, stop=True)
            nc.scalar.activation(out=aggT[:, csl], in_=ps[:],
                                 func=AF.Identity, bias=b2x32[:], scale=1.0)
            for fh in range(2):
                fsl = slice(fh * D, (fh + 1) * D)
                ps2 = psC.tile([D, 512], F32, tag="h2p", bufs=2)
                nc.tensor.matmul(ps2[:], fw1[:, 0, fsl], sT[:, csl],
                                 start=True, stop=False)
                nc.tensor.matmul(ps2[:], fw1[:, 1, fsl], aggT[:, csl],
                                 start=False, stop=True)
                c = fh * 2 + nh
                nc.scalar.activation(out=H2[fh][:, csl], in_=ps2[:],
                                     func=AF.Copy,
                                     accum_out=st2sum[:, c:c + 1])
                sq2 = statp.tile([D, 512], F32, tag="sq2")
                nc.vector.scalar_tensor_tensor(
                    out=sq2[:], in0=H2[fh][:, csl], scalar=1.0,
                    in1=H2[fh][:, csl], op0=ALU.mult, op1=ALU.mult,
                    accum_out=st2sq[:, c:c + 1])

        stat2 = statp.tile([D, 4], F32)
        nc.vector.reduce_sum(stat2[:, 0:2],
                             st2sum[:].rearrange("p (fh nh) -> p fh nh", nh=2),
                             axis=mybir.AxisListType.X)
        nc.vector.reduce_sum(stat2[:, 2:4],
                             st2sq[:].rearrange("p (fh nh) -> p fh nh", nh=2),
                             axis=mybir.AxisListType.X)
        cc2_in = dram.tile([D, 4], F32, tag="cc2i")
        cc2_out = dram.tile([D, 4], F32, tag="cc2o")
        nc.sync.dma_start(out=cc2_in[:], in_=stat2[:])
        if NO_CC:
            nc.sync.dma_start(out=cc2_out[:], in_=cc2_in[:])
        else:
            nc.gpsimd.collective_compute(
                "AllReduce", ALU.add, replica_groups=[list(range(NCORES))],
                ins=[cc2_in[:].opt()], outs=[cc2_out[:].opt()])
        statg2 = statp.tile([D, 4], F32)
        nc.sync.dma_start(out=statg2[:], in_=cc2_out[:])
        a2, z2 = bn_coeffs(statg2, N2 / (NCORES if NO_CC else 1), g2c, be2c,
                           statp, "bn2_")

        # ------------- m2 = Prelu(a2*H2+z2); outT = FW2 @ m2 + fb2 ---------
        m2 = [big.tile([D, ROWS], BF16, tag="m2", bufs=2, name=f"m2_{h}")
              for h in range(2)]
        for fh in range(2):
            nc.scalar.activation(out=m2[fh][:], in_=H2[fh][:], func=AF.Prelu,
                                 scale=a2[:, fh:fh + 1], bias=z2[:, fh:fh + 1],
                                 alpha=SLOPE)
        outT = statp.tile([D, ROWS], F32, name="outT")
        for nh in range(2):
            csl = slice(nh * 512, (nh + 1) * 512)
            ps = psC.tile([D, 512], F32, tag="outp", bufs=2)
            nc.tensor.matmul(ps[:], fw2[:, 0, :], m2[0][:, csl],
                             start=True, stop=False)
            nc.tensor.matmul(ps[:], fw2[:, 1, :], m2[1][:, csl],
                             start=False, stop=True)
            nc.scalar.activation(out=outT[:, csl], in_=ps[:],
                                 func=AF.Identity, bias=fb2c[:], scale=1.0)
        nc.sync.dma_start(out=outT_d.ap(), in_=outT[:])
    return nc


def _build_nc_staged():
    nc = _build_nc()
    nc.compile()
    return nc


_NC_CACHE = {}


def _get_nc():
    if "nc" not in _NC_CACHE:
        _NC_CACHE["nc"] = _build_nc_staged()
    return _NC_CACHE["nc"]


def _prep_in_maps(state, edges, msg_w1, msg_b1, msg_gamma, msg_beta, msg_w2,
                  msg_b2, fin_w1, fin_b1, fin_gamma, fin_beta, fin_w2, fin_b2,
                  **_unused):
    f32 = np.float32
    state = np.asarray(state, f32)
    edges = np.asarray(edges, f32)

    shared = {
        "w1aT": np.ascontiguousarray(np.asarray(msg_w1, f32)[:, :D].T).astype(NPBF),
        "w1bT": np.ascontiguousarray(np.asarray(msg_w1, f32)[:, D:].T).astype(NPBF),
        "w2T": np.ascontiguousarray(np.asarray(msg_w2, f32).T),
        "fw1T": np.ascontiguousarray(np.asarray(fin_w1, f32).T).astype(NPBF),
        "fw2T": np.ascontiguousarray(np.asarray(fin_w2, f32).T).astype(NPBF),
        "g1": np.ascontiguousarray(np.asarray(msg_gamma, f32)),
        "be1": np.ascontiguousarray(np.asarray(msg_beta, f32)),
        "b2": np.ascontiguousarray(np.asarray(msg_b2, f32)),
        "g2": np.ascontiguousarray(np.asarray(fin_gamma, f32)),
        "be2": np.ascontiguousarray(np.asarray(fin_beta, f32)),
        "fb2": np.ascontiguousarray(np.asarray(fin_b2, f32)),
    }
    idx = np.arange(NOBJ)
    in_maps = []
    for c in range(NCORES):
        sh = state[c * NB:(c + 1) * NB].reshape(ROWS, D)
        ed = edges[c * NB:(c + 1) * NB]          # [32, 1024]
        em = ed.reshape(NB, NOBJ, NOBJ)          # [b, i, j]
        deg = em.sum(axis=2)                     # [b, i]
        cdeg = em.sum(axis=1)                    # [b, j]
        # pq: [128, s, 2048]: cols = [b0(1024) | b1(1024)], partitions
        # [0:32) P(b0) on b0-cols, [32:64) P(b1) on b1-cols, [64:96) Q(b0),
        # [96:128) Q(b1); zero elsewhere, so one K=128 matmul computes both
        # batches.  uvT/wdeg use the matching [u_b0; u_b1; v_b0; v_b1] rows.
        pq = np.zeros((D, NS, 2, NOBJ * NOBJ), f32)
        wdeg = np.empty((D, NS), f32)
        for s in range(NS):
            for half in range(2):
                b = 2 * s + half
                P3 = pq[32 * half:32 * (half + 1), s, half].reshape(
                    NOBJ, NOBJ, NOBJ)
                P3[idx, idx, :] = em[b]          # P[k,k,:] = e[b,k,:]
                Q3 = pq[64 + 32 * half:64 + 32 * (half + 1), s, half].reshape(
                    NOBJ, NOBJ, NOBJ)
                Q3[idx, :, idx] = em[b].T        # Q[k,:,k] = e[b,:,k]
                wdeg[32 * half:32 * (half + 1), s] = deg[b]
                wdeg[64 + 32 * half:64 + 32 * (half + 1), s] = cdeg[b]
        in_maps.append({
            "stateT": np.ascontiguousarray(sh.T).astype(NPBF),
            "state_rm": np.ascontiguousarray(sh).astype(NPBF),
            "pq": np.ascontiguousarray(pq.reshape(D, -1)).astype(NPBF),
            "wdeg": np.ascontiguousarray(wdeg).astype(NPBF),
            "edgesT_s": np.ascontiguousarray(
                em.transpose(0, 2, 1).reshape(NB, -1)).astype(NPBF),
            **shared,
        })
    return in_maps


def kernel(**inputs):
    in_maps = _prep_in_maps(**inputs)
    nc = _get_nc()
    res = run_bass_kernel_spmd(nc, in_maps, core_ids=list(range(NCORES)))
    out = np.empty((B, NOBJ, D), np.float32)
    for c in range(NCORES):
        outT = res.results[c]["outT"]                       # [128, 1024]
        out[c * NB:(c + 1) * NB] = outT.T.reshape(NB, NOBJ, D)
    return out


if __name__ == "__main__":
    print("smoke-building nc...")
    _get_nc()
    print("built OK")


# revision 18
# speedup vs baseline: 1.9250x; 1.1832x over previous
"""Trainium2 Bass kernel for nn_DiscretePredictor (gnn_message_passing), v2.

Reference (per batch b of 256, n=32 objects, d=128):
    e = edges[b, i*n+j] in {0,1}
    msgs = MLP1([state_i*e, state_j*e]) : Lin(256->256) -> BN -> LReLU -> Lin(256->128)
    agg[b,i] = sum_j msgs ;  out = MLP2([state, agg]) same shape.

v2 strategy (vs v1 which did the masked pre-activation elementwise on DVE/Pool):
  * H = e*(u_i + v_j) is built ON THE PE as one K=64 matmul per (batch,
    f-half): lhsT = [u_b rows; v_b rows] (32+32 partitions), rhs = PQ_b, a
    host-precomputed block-diagonal operand with P[k,(i,j)] = e[i,j]*d(k==i)
    (routes+masks u) stacked on Q[k,(i,j)] = e[i,j]*d(k==j) (routes+masks v).
    This deletes the two 34-65us elementwise passes (add + mask) entirely.
  * All matmuls run in bf16 (1 cyc/row vs 4 for fp32).
  * BN1 scale/shift are applied inside the Prelu activation (scale=a1,
    bias=z1 per-partition APs) - no re-matmul with folded weights.
  * BN1 stats come from tiny PE matmuls: sum(H) = sum deg*u + cdeg*v and
    sum(H^2) = sum deg*u^2 + cdeg*v^2 + 2*w1a^T M w1b with M = sum S^T E S,
    using a [128,16] host-built deg/cdeg vector matched to the uv layout.
  * The j-reduction is a packed-bf16 halving tree on DVE (2x mode) for some
    units and a straight Pool tensor_reduce for the rest; a few Prelu units
    run on DVE (tensor_scalar fused a1*H+z1, then max(t, slope*t)) to
    balance the three engines.

Sharding: data-parallel over batch (32 batches/core), params replicated,
two [128,4] AllReduces for the sync-BN stats.
"""

import os
import sys

for p in ("/opt/trn_rl_repo", "/root/.axon_site", "/root/.axon_site/_ro/trn_rl_repo",
          "/root/.axon_site/_ro/pypackages"):
    if os.path.isdir(p) and p not in sys.path:
        sys.path.append(p)

import numpy as np
import ml_dtypes

import concourse.bass as bass
import concourse.mybir as mybir
import concourse.tile as tile
from concourse import bacc
from concourse.bass_utils import run_bass_kernel_spmd

F32 = mybir.dt.float32
BF16 = mybir.dt.bfloat16
AF = mybir.ActivationFunctionType
ALU = mybir.AluOpType
NPBF = ml_dtypes.bfloat16

B = 256          # global batch
NOBJ = 32        # objects per batch
D = 128          # object dim
F = 256          # hidden width (both MLPs)
NCORES = 8
NB = B // NCORES          # batches per core = 32
ROWS = NB * NOBJ          # (b,i) rows per core = 1024
NS = NB // 2              # pair-slots (2 batches each) = 16
N1 = float(B * NOBJ * NOBJ)   # BN1 row count (global)
N2 = float(B * NOBJ)          # BN2 row count (global)
EPS = 1e-5
SLOPE = 0.01
NO_CC = os.environ.get("BASS_NO_CC", "0") == "1"

# engine-balance knobs: units are (s, fh), 32 total
DVE_PRELU = int(os.environ.get("BASS_DVE_PRELU", "5"))    # units preluing on DVE
POOL_RED = int(os.environ.get("BASS_POOL_RED", "22"))     # units whose tree-L1 runs on Pool


def _unit_flags():
    """Spread DVE-prelu and Pool-reduce units evenly over the 32 units."""
    units = [(s, fh) for s in range(NS) for fh in range(2)]
    n = len(units)
    dve_p = set()
    if DVE_PRELU > 0:
        step = n / DVE_PRELU
        dve_p = {units[min(n - 1, int(i * step))] for i in range(DVE_PRELU)}
    pool_r = set()
    if POOL_RED > 0:
        step = n / POOL_RED
        pool_r = {units[min(n - 1, int(i * step + 0.5))] for i in range(POOL_RED)}
    return dve_p, pool_r


def _build_nc():
    nc = bacc.Bacc("TRN2", target_bir_lowering=False, debug=False,
                   enable_asserts=True, num_devices=NCORES)

    # ---- per-core device I/O ----
    sT_d = nc.dram_tensor("stateT", [D, ROWS], BF16, kind="ExternalInput")
    pq_d = nc.dram_tensor("pq", [D, NS * 2 * NOBJ * NOBJ], BF16, kind="ExternalInput")
    wdeg_d = nc.dram_tensor("wdeg", [D, NS], BF16, kind="ExternalInput")
    ea_d = nc.dram_tensor("eafull", [2 * NOBJ, NS * 2 * NOBJ], BF16, kind="ExternalInput")
    w1a_d = nc.dram_tensor("w1aT", [D, F], BF16, kind="ExternalInput")
    w1b_d = nc.dram_tensor("w1bT", [D, F], BF16, kind="ExternalInput")
    w2k_d = nc.dram_tensor("w2T", [F, D], F32, kind="ExternalInput")
    fw1_d = nc.dram_tensor("fw1T", [2 * D, F], BF16, kind="ExternalInput")
    fw2_d = nc.dram_tensor("fw2T", [F, D], BF16, kind="ExternalInput")
    g1_d = nc.dram_tensor("g1", [F], F32, kind="ExternalInput")
    be1_d = nc.dram_tensor("be1", [F], F32, kind="ExternalInput")
    b2_d = nc.dram_tensor("b2", [D], F32, kind="ExternalInput")
    g2_d = nc.dram_tensor("g2", [F], F32, kind="ExternalInput")
    be2_d = nc.dram_tensor("be2", [F], F32, kind="ExternalInput")
    fb2_d = nc.dram_tensor("fb2", [D], F32, kind="ExternalInput")
    outT_d = nc.dram_tensor("outT", [D, ROWS], F32, kind="ExternalOutput")

    dve_prelu_units, pool_red_units = _unit_flags()

    from contextlib import ExitStack
    with tile.TileContext(nc) as tc, ExitStack() as ctx:
        consts = ctx.enter_context(tc.tile_pool(name="consts", bufs=1))
        uvp = ctx.enter_context(tc.tile_pool(name="uv", bufs=1))
        big = ctx.enter_context(tc.tile_pool(name="big", bufs=2))
        statp = ctx.enter_context(tc.tile_pool(name="stats", bufs=1))
        dram = ctx.enter_context(tc.tile_pool(name="dram", bufs=1, space="DRAM"))

        # ---------------- setup: load params + state ----------------
        sT = consts.tile([D, ROWS], BF16)
        nc.sync.dma_start(out=sT[:], in_=sT_d.ap())
        wdeg = consts.tile([D, NS], BF16)
        nc.sync.dma_start(out=wdeg[:], in_=wdeg_d.ap())
        # block-diag E pair-matrices for the cross-term
        eafull = consts.tile([2 * NOBJ, NS, 2 * NOBJ], BF16)
        nc.sync.dma_start(out=eafull[:],
                          in_=ea_d.ap().rearrange("p (s c) -> p s c", s=NS))
        # pq is large (64KB/partition): split across the three HWDGE queues
        # so it never blocks the small latency-critical loads on sync.
        pqt = consts.tile([D, NS, 2 * NOBJ * NOBJ], BF16)
        pq_src = pq_d.ap().rearrange("p (s c) -> p s c", s=NS)
        for q, eng in enumerate((nc.scalar, nc.gpsimd, nc.scalar, nc.gpsimd)):
            eng.dma_start(out=pqt[:, 4 * q:4 * (q + 1), :],
                          in_=pq_src[:, 4 * q:4 * (q + 1), :])
        w1a = consts.tile([D, F], BF16)
        w1b = consts.tile([D, F], BF16)
        nc.sync.dma_start(out=w1a[:], in_=w1a_d.ap())
        nc.sync.dma_start(out=w1b[:], in_=w1b_d.ap())
        w2k = consts.tile([D, 2, D], F32)
        fw1 = consts.tile([D, 2, F], BF16)
        fw2 = consts.tile([D, 2, D], BF16)
        nc.sync.dma_start(out=w2k[:], in_=w2k_d.ap().rearrange("(k p) d -> p k d", p=D))
        nc.sync.dma_start(out=fw1[:], in_=fw1_d.ap().rearrange("(k p) f -> p k f", p=D))
        nc.sync.dma_start(out=fw2[:], in_=fw2_d.ap().rearrange("(k p) d -> p k d", p=D))

        def fvec(dh, nm):  # [256] dram vector -> [128, 2] feature-major sbuf
            t = consts.tile([D, 2], F32, tag=nm, name=nm)
            nc.sync.dma_start(out=t[:], in_=dh.ap().rearrange("(h p) -> p h", p=D))
            return t

        def dvec(dh, nm):  # [128] -> [128, 1]
            t = consts.tile([D, 1], F32, tag=nm, name=nm)
            nc.sync.dma_start(out=t[:], in_=dh.ap().rearrange("(h p) -> p h", p=D))
            return t

        g1c, be1c = fvec(g1_d, "g1c"), fvec(be1_d, "be1c")
        g2c, be2c = fvec(g2_d, "g2c"), fvec(be2_d, "be2c")
        b2c, fb2c = dvec(b2_d, "b2c"), dvec(fb2_d, "fb2c")
        b2x32 = consts.tile([D, 1], F32)
        nc.vector.tensor_scalar_mul(b2x32[:], b2c[:], float(NOBJ))
        ones1 = consts.tile([D, 1], BF16)
        nc.vector.memset(ones1[:], 1.0)
        # warm the ACT function tables (Prelu + Copy/Square/Sqrt set) while
        # the engines idle on input DMAs
        dum = consts.tile([1, 2], F32, name="dum")
        nc.vector.memset(dum[:], 1.0)
        dum2 = consts.tile([1, 2], F32, name="dum2")
        nc.scalar.activation(out=dum2[:], in_=dum[:], func=AF.Prelu, alpha=SLOPE)
        nc.scalar.activation(out=dum2[:], in_=dum[:], func=AF.Sqrt)

        # ------------- uvT matmuls: interleaved [u_b0; v_b0; u_b1; v_b1] ----
        # uvT[p, s, f]: p in [0,32) u rows of batch 2s (i=p), [32,64) v rows
        # of 2s, [64,96) u of 2s+1, [96,128) v of 2s+1.
        uvT = uvp.tile([D, NS, F], BF16)
        uvSq = uvp.tile([D, NS, F], BF16)
        sc1 = statp.tile([1, 2 * F], F32)
        psA_cm = tc.tile_pool(name="psA", bufs=2, space="PSUM")
        psA = psA_cm.__enter__()
        for s in range(NS):
            ps = psA.tile([D, F], F32, tag="uvps", bufs=2)
            rsl = slice(2 * s * NOBJ, (2 * s + 2) * NOBJ)   # 64 rows: b0|b1
            nc.tensor.matmul(ps[0:64, :], sT[:, rsl], w1a[:],
                             start=True, stop=True)
            nc.tensor.matmul(ps[64:128, :], sT[:, rsl], w1b[:],
                             start=True, stop=True)
            nc.scalar.activation(out=uvT[:, s, :], in_=ps[:], func=AF.Copy)
        with nc.allow_low_precision(reason="bf16 squares feed fp32 psum sums"):
            for h in range(2):
                nc.vector.tensor_mul(uvSq[:, 8 * h:8 * (h + 1), :],
                                     uvT[:, 8 * h:8 * (h + 1), :],
                                     uvT[:, 8 * h:8 * (h + 1), :])

        # ------------- BN1 stats: sum(H), sum(H^2) ----------------
        # sum(H)[f]  = sum_s sum_p wdeg[p,s]*uvT[p,s,f]   (deg for u rows,
        # cdeg for v rows, both batches)  -> [1, 256] psum accumulate
        shp = psA.tile([1, F], F32, tag="shp", bufs=1, name="shp")
        sqp = psA.tile([1, F], F32, tag="sqp", bufs=1, name="sqp")
        for s in range(NS):
            nc.tensor.matmul(shp[:], wdeg[:, s:s + 1], uvT[:, s, :],
                             start=(s == 0), stop=(s == NS - 1))
        for s in range(NS):
            nc.tensor.matmul(sqp[:], wdeg[:, s:s + 1], uvSq[:, s, :],
                             start=(s == 0), stop=(s == NS - 1))

        # cross term: sum e*u*v = sum_{b,i,f} u[(b,i),f] * (E_b v_b)[i,f].
        # ebv per slot lands on partitions [0:64) (i rows of b0|b1); multiply
        # by the matching u rows of uvT and column-sum via a ones-matmul.
        tm = uvp.tile([D, NS, F], BF16)
        crp = psA.tile([1, F], F32, tag="crp", bufs=1, name="crp")
        for s in range(NS):
            # ebtu[64+32h+j] = (E_b^T u_b)[j], landing on v_b's partitions
            ebtu = psA.tile([D, F], F32, tag="ebv", bufs=2)
            nc.tensor.matmul(ebtu[64:128, :], eafull[:, s, :],
                             uvT[0:64, s, :], start=True, stop=True)
            with nc.allow_low_precision(reason="cross-term partial"):
                nc.vector.tensor_mul(tm[64:128, s, :], uvT[64:128, s, :],
                                     ebtu[64:128, :])
        for s in range(NS):
            nc.tensor.matmul(crp[:], ones1[64:128, :], tm[64:128, s, :],
                             start=(s == 0), stop=(s == NS - 1))

        # pack [sumH(256) | sumH2(256)] on one partition, roundtrip via DRAM
        # to feature-major [128, 4], AllReduce, then BN coeffs.
        nc.scalar.activation(out=sc1[:, 0:F], in_=shp[:], func=AF.Copy)
        nc.vector.scalar_tensor_tensor(out=sc1[:, F:2 * F], in0=crp[:], scalar=2.0,
                                       in1=sqp[:], op0=ALU.mult, op1=ALU.add)
        cc1_in = dram.tile([1, 2 * F], F32, tag="cc1i")
        nc.sync.dma_start(out=cc1_in[:], in_=sc1[:])
        cc1_out = dram.tile([1, 2 * F], F32, tag="cc1o")
        if NO_CC:
            nc.sync.dma_start(out=cc1_out[:], in_=cc1_in[:])
        else:
            nc.gpsimd.collective_compute(
                "AllReduce", ALU.add, replica_groups=[list(range(NCORES))],
                ins=[cc1_in[:].opt()], outs=[cc1_out[:].opt()])
        statg1 = statp.tile([D, 4], F32)
        nc.sync.dma_start(out=statg1[:],
                          in_=cc1_out[:].rearrange("x (g h p) -> (x p) (g h)",
                                                   p=D, g=2))

        epsc = statp.tile([D, 1], F32)
        nc.vector.memset(epsc[:], EPS)

        def bn_coeffs(statg, n_rows, gc, bec, pool, pre):
            def tl(nm):
                return pool.tile([D, 2], F32, tag=pre + nm, name=pre + nm)
            mean, msq, var = tl("mean"), tl("msq"), tl("var")
            sd, rstd, a, ma, z = tl("sd"), tl("rstd"), tl("a"), tl("ma"), tl("z")
            nc.scalar.activation(out=mean[:], in_=statg[:, 0:2], func=AF.Copy,
                                 scale=1.0 / n_rows)
            nc.scalar.activation(out=msq[:], in_=mean[:], func=AF.Square)
            nc.vector.scalar_tensor_tensor(
                out=var[:], in0=statg[:, 2:4], scalar=1.0 / n_rows,
                in1=msq[:], op0=ALU.mult, op1=ALU.subtract)
            nc.scalar.activation(out=sd[:], in_=var[:], func=AF.Sqrt, bias=epsc[:])
            nc.vector.reciprocal(out=rstd[:], in_=sd[:])
            nc.vector.tensor_mul(a[:], gc[:], rstd[:])
            nc.vector.tensor_mul(ma[:], mean[:], a[:])
            nc.vector.tensor_sub(z[:], bec[:], ma[:])
            return a, z

        a1, z1 = bn_coeffs(statg1, N1 / (NCORES if NO_CC else 1), g1c, be1c,
                           statp, "bn1_")

        # ------------- big phase: H on PE, Prelu, j-reduce ----------------
        # per unit (s, fh): H[128, 2048] = [b0 cols | b1 cols] in PSUM,
        #   m = Prelu(a1*H + z1)  (ACT, or DVE 2-instr for some units)
        #   msum[:, s*64:(s+1)*64] = sum_j m  (DVE halving tree or Pool)
        msumS = [statp.tile([D, ROWS], F32, tag=f"msum{h}", name=f"msum{h}")
                 for h in range(2)]
        psA_cm.__exit__(None, None, None)  # release psA banks for the big phase
        psB_cm = tc.tile_pool(name="psB", bufs=2, space="PSUM")
        psB = psB_cm.__enter__()
        CG2 = 2 * NOBJ * NOBJ  # 2048
        for s in range(NS):
            for fh in range(2):
                hps = psB.tile([D, CG2], F32, tag="hps", bufs=2)
                nc.tensor.matmul(hps[:], uvT[:, s, fh * D:(fh + 1) * D],
                                 pqt[:, s, :], start=True, stop=True)
                mt = big.tile([D, CG2], BF16, tag="mt", bufs=4, name=f"mt{s}{fh}")
                if (s, fh) in dve_prelu_units:
                    tf = big.tile([D, CG2], F32, tag="tf", bufs=3, name=f"tf{s}{fh}")
                    nc.vector.tensor_scalar(
                        out=tf[:], in0=hps[:], scalar1=a1[:, fh:fh + 1],
                        scalar2=z1[:, fh:fh + 1], op0=ALU.mult, op1=ALU.add)
                    with nc.allow_low_precision(reason="prelu out"):
                        nc.vector.scalar_tensor_tensor(
                            out=mt[:], in0=tf[:], scalar=SLOPE, in1=tf[:],
                            op0=ALU.mult, op1=ALU.max)
                else:
                    nc.scalar.activation(out=mt[:], in_=hps[:], func=AF.Prelu,
                                         scale=a1[:, fh:fh + 1],
                                         bias=z1[:, fh:fh + 1], alpha=SLOPE)
                osl = msumS[fh][:, s * 64:(s + 1) * 64]
                # packed-bf16 halving tree over j (2x DVE mode); level 1 may
                # run on Pool to balance engines
                cur = mt[:].rearrange("p (r j) -> p r j", j=NOBJ)
                w = NOBJ
                with nc.allow_low_precision(reason="bf16 j-tree"):
                    while w > 2:
                        w //= 2
                        nt = big.tile([D, 64 * w], BF16, tag=f"tr{w}",
                                      bufs=4, name=f"tr{s}{fh}{w}")
                        nv = nt[:].rearrange("p (r j) -> p r j", j=w)
                        eng = (nc.gpsimd if (w == NOBJ // 2 and
                                             (s, fh) in pool_red_units)
                               else nc.vector)
                        eng.tensor_add(nv, cur[:, :, 0:w], cur[:, :, w:2 * w])
                        cur = nv
                nc.vector.tensor_add(osl.rearrange("p (r x) -> p r x", x=1),
                                     cur[:, :, 0:1], cur[:, :, 1:2])

        # ------------- aggT = W2 @ msum + 32*b2 ; H2 = FW1 @ [sT; aggT] -----
        psB_cm.__exit__(None, None, None)
        psC = ctx.enter_context(tc.tile_pool(name="psC", bufs=2, space="PSUM"))
        aggT = statp.tile([D, ROWS], BF16, name="aggT")
        H2 = [statp.tile([D, ROWS], F32, tag=f"h2_{h}", name=f"h2_{h}")
              for h in range(2)]
        st2sum = statp.tile([D, 4], F32)   # col = fh*2 + nh : sum H2
        st2sq = statp.tile([D, 4], F32)    # col = fh*2 + nh : sum H2^2
        for nh in range(2):
            csl = slice(nh * 512, (nh + 1) * 512)
            ps = psC.tile([D, 512], F32, tag="aggp", bufs=2)
            nc.tensor.matmul(ps[:], w2k[:, 0, :], msumS[0][:, csl],
                             start=True, stop=False)
            nc.tensor.matmul(ps[:], w2k[:, 1, :], msumS[1][:, csl],
                             start=False, stop=True)
            nc.scalar.activation(out=aggT[:, csl], in_=ps[:],
                                 func=AF.Identity, bias=b2x32[:], scale=1.0)
            for fh in range(2):
                fsl = slice(fh * D, (fh + 1) * D)
                ps2 = psC.tile([D, 512], F32, tag="h2p", bufs=2)
                nc.tensor.matmul(ps2[:], fw1[:, 0, fsl], sT[:, csl],
                                 start=True, stop=False)
                nc.tensor.matmul(ps2[:], fw1[:, 1, fsl], aggT[:, csl],
                                 start=False, stop=True)
                c = fh * 2 + nh
                nc.scalar.activation(out=H2[fh][:, csl], in_=ps2[:],
                                     func=AF.Copy,
                                     accum_out=st2sum[:, c:c + 1])
                sq2 = statp.tile([D, 512], F32, tag="sq2")
                nc.vector.scalar_tensor_tensor(
                    out=sq2[:], in0=H2[fh][:, csl], scalar=1.0,
                    in1=H2[fh][:, csl], op0=ALU.mult, op1=ALU.mult,
                    accum_out=st2sq[:, c:c + 1])

        stat2 = statp.tile([D, 4], F32)
        nc.vector.reduce_sum(stat2[:, 0:2],
                             st2sum[:].rearrange("p (fh nh) -> p fh nh", nh=2),
                             axis=mybir.AxisListType.X)
        nc.vector.reduce_sum(stat2[:, 2:4],
                             st2sq[:].rearrange("p (fh nh) -> p fh nh", nh=2),
                             axis=mybir.AxisListType.X)
        cc2_in = dram.tile([D, 4], F32, tag="cc2i")
        cc2_out = dram.tile([D, 4], F32, tag="cc2o")
        nc.sync.dma_start(out=cc2_in[:], in_=stat2[:])
        if NO_CC:
            nc.sync.dma_start(out=cc2_out[:], in_=cc2_in[:])
        else:
            nc.gpsimd.collective_compute(
                "AllReduce", ALU.add, replica_groups=[list(range(NCORES))],
                ins=[cc2_in[:].opt()], outs=[cc2_out[:].opt()])
        statg2 = statp.tile([D, 4], F32)
        nc.sync.dma_start(out=statg2[:], in_=cc2_out[:])
        a2, z2 = bn_coeffs(statg2, N2 / (NCORES if NO_CC else 1), g2c, be2c,
                           statp, "bn2_")

        # ------------- m2 = Prelu(a2*H2+z2); outT = FW2 @ m2 + fb2 ---------
        m2 = [big.tile([D, ROWS], BF16, tag="m2", bufs=2, name=f"m2_{h}")
              for h in range(2)]
        for fh in range(2):
            nc.scalar.activation(out=m2[fh][:], in_=H2[fh][:], func=AF.Prelu,
                                 scale=a2[:, fh:fh + 1], bias=z2[:, fh:fh + 1],
                                 alpha=SLOPE)
        outT = statp.tile([D, ROWS], F32, name="outT")
        for nh in range(2):
            csl = slice(nh * 512, (nh + 1) * 512)
            ps = psC.tile([D, 512], F32, tag="outp", bufs=2)
            nc.tensor.matmul(ps[:], fw2[:, 0, :], m2[0][:, csl],
                             start=True, stop=False)
            nc.tensor.matmul(ps[:], fw2[:, 1, :], m2[1][:, csl],
                             start=False, stop=True)
            nc.scalar.activation(out=outT[:, csl], in_=ps[:],
                                 func=AF.Identity, bias=fb2c[:], scale=1.0)
        nc.sync.dma_start(out=outT_d.ap(), in_=outT[:])
    return nc


def _build_nc_staged():
    nc = _build_nc()
    nc.compile()
    return nc


_NC_CACHE = {}


def _get_nc():
    if "nc" not in _NC_CACHE:
        _NC_CACHE["nc"] = _build_nc_staged()
    return _NC_CACHE["nc"]


def _prep_in_maps(state, edges, msg_w1, msg_b1, msg_gamma, msg_beta, msg_w2,
                  msg_b2, fin_w1, fin_b1, fin_gamma, fin_beta, fin_w2, fin_b2,
                  **_unused):
    f32 = np.float32
    state = np.asarray(state, f32)
    edges = np.asarray(edges, f32)

    shared = {
        "w1aT": np.ascontiguousarray(np.asarray(msg_w1, f32)[:, :D].T).astype(NPBF),
        "w1bT": np.ascontiguousarray(np.asarray(msg_w1, f32)[:, D:].T).astype(NPBF),
        "w2T": np.ascontiguousarray(np.asarray(msg_w2, f32).T),
        "fw1T": np.ascontiguousarray(np.asarray(fin_w1, f32).T).astype(NPBF),
        "fw2T": np.ascontiguousarray(np.asarray(fin_w2, f32).T).astype(NPBF),
        "g1": np.ascontiguousarray(np.asarray(msg_gamma, f32)),
        "be1": np.ascontiguousarray(np.asarray(msg_beta, f32)),
        "b2": np.ascontiguousarray(np.asarray(msg_b2, f32)),
        "g2": np.ascontiguousarray(np.asarray(fin_gamma, f32)),
        "be2": np.ascontiguousarray(np.asarray(fin_beta, f32)),
        "fb2": np.ascontiguousarray(np.asarray(fin_b2, f32)),
    }
    idx = np.arange(NOBJ)
    in_maps = []
    for c in range(NCORES):
        sh = state[c * NB:(c + 1) * NB].reshape(ROWS, D)
        ed = edges[c * NB:(c + 1) * NB]          # [32, 1024]
        em = ed.reshape(NB, NOBJ, NOBJ)          # [b, i, j]
        deg = em.sum(axis=2)                     # [b, i]
        cdeg = em.sum(axis=1)                    # [b, j]
        # pq: [128, s, 2048]: cols = [b0(1024) | b1(1024)], partitions
        # [0:32) P(b0) on b0-cols, [32:64) P(b1) on b1-cols, [64:96) Q(b0),
        # [96:128) Q(b1); zero elsewhere, so one K=128 matmul computes both
        # batches.  uvT/wdeg use the matching [u_b0; u_b1; v_b0; v_b1] rows.
        pq = np.zeros((D, NS, 2, NOBJ * NOBJ), f32)
        wdeg = np.empty((D, NS), f32)
        ea = np.zeros((2 * NOBJ, NS, 2 * NOBJ), f32)
        for s in range(NS):
            for half in range(2):
                b = 2 * s + half
                P3 = pq[32 * half:32 * (half + 1), s, half].reshape(
                    NOBJ, NOBJ, NOBJ)
                P3[idx, idx, :] = em[b]          # P[k,k,:] = e[b,k,:]
                Q3 = pq[64 + 32 * half:64 + 32 * (half + 1), s, half].reshape(
                    NOBJ, NOBJ, NOBJ)
                Q3[idx, :, idx] = em[b].T        # Q[k,:,k] = e[b,:,k]
                wdeg[32 * half:32 * (half + 1), s] = deg[b]
                wdeg[64 + 32 * half:64 + 32 * (half + 1), s] = cdeg[b]
                ea[32 * half:32 * (half + 1), s,
                   32 * half:32 * (half + 1)] = em[b]
        in_maps.append({
            "stateT": np.ascontiguousarray(sh.T).astype(NPBF),
            "pq": np.ascontiguousarray(pq.reshape(D, -1)).astype(NPBF),
            "wdeg": np.ascontiguousarray(wdeg).astype(NPBF),
            "eafull": np.ascontiguousarray(ea.reshape(2 * NOBJ, -1)).astype(NPBF),
            **shared,
        })
    return in_maps


def kernel(**inputs):
    in_maps = _prep_in_maps(**inputs)
    nc = _get_nc()
    res = run_bass_kernel_spmd(nc, in_maps, core_ids=list(range(NCORES)))
    out = np.empty((B, NOBJ, D), np.float32)
    for c in range(NCORES):
        outT = res.results[c]["outT"]                       # [128, 1024]
        out[c * NB:(c + 1) * NB] = outT.T.reshape(NB, NOBJ, D)
    return out


if __name__ == "__main__":
    print("smoke-building nc...")
    _get_nc()
    print("built OK")


# revision 19
# speedup vs baseline: 1.9621x; 1.0193x over previous
"""Trainium2 Bass kernel for nn_DiscretePredictor (gnn_message_passing), v2.

Reference (per batch b of 256, n=32 objects, d=128):
    e = edges[b, i*n+j] in {0,1}
    msgs = MLP1([state_i*e, state_j*e]) : Lin(256->256) -> BN -> LReLU -> Lin(256->128)
    agg[b,i] = sum_j msgs ;  out = MLP2([state, agg]) same shape.

v2 strategy (vs v1 which did the masked pre-activation elementwise on DVE/Pool):
  * H = e*(u_i + v_j) is built ON THE PE as one K=64 matmul per (batch,
    f-half): lhsT = [u_b rows; v_b rows] (32+32 partitions), rhs = PQ_b, a
    host-precomputed block-diagonal operand with P[k,(i,j)] = e[i,j]*d(k==i)
    (routes+masks u) stacked on Q[k,(i,j)] = e[i,j]*d(k==j) (routes+masks v).
    This deletes the two 34-65us elementwise passes (add + mask) entirely.
  * All matmuls run in bf16 (1 cyc/row vs 4 for fp32).
  * BN1 scale/shift are applied inside the Prelu activation (scale=a1,
    bias=z1 per-partition APs) - no re-matmul with folded weights.
  * BN1 stats come from tiny PE matmuls: sum(H) = sum deg*u + cdeg*v and
    sum(H^2) = sum deg*u^2 + cdeg*v^2 + 2*w1a^T M w1b with M = sum S^T E S,
    using a [128,16] host-built deg/cdeg vector matched to the uv layout.
  * The j-reduction is a packed-bf16 halving tree on DVE (2x mode) for some
    units and a straight Pool tensor_reduce for the rest; a few Prelu units
    run on DVE (tensor_scalar fused a1*H+z1, then max(t, slope*t)) to
    balance the three engines.

Sharding: data-parallel over batch (32 batches/core), params replicated,
two [128,4] AllReduces for the sync-BN stats.
"""

import os
import sys

for p in ("/opt/trn_rl_repo", "/root/.axon_site", "/root/.axon_site/_ro/trn_rl_repo",
          "/root/.axon_site/_ro/pypackages"):
    if os.path.isdir(p) and p not in sys.path:
        sys.path.append(p)

import numpy as np
import ml_dtypes

import concourse.bass as bass
import concourse.mybir as mybir
import concourse.tile as tile
from concourse import bacc
from concourse.bass_utils import run_bass_kernel_spmd

F32 = mybir.dt.float32
BF16 = mybir.dt.bfloat16
AF = mybir.ActivationFunctionType
ALU = mybir.AluOpType
NPBF = ml_dtypes.bfloat16

B = 256          # global batch
NOBJ = 32        # objects per batch
D = 128          # object dim
F = 256          # hidden width (both MLPs)
NCORES = 8
NB = B // NCORES          # batches per core = 32
ROWS = NB * NOBJ          # (b,i) rows per core = 1024
NS = NB // 2              # pair-slots (2 batches each) = 16
N1 = float(B * NOBJ * NOBJ)   # BN1 row count (global)
N2 = float(B * NOBJ)          # BN2 row count (global)
EPS = 1e-5
SLOPE = 0.01
NO_CC = os.environ.get("BASS_NO_CC", "0") == "1"

# engine-balance knobs: units are (s, fh), 32 total
DVE_PRELU = int(os.environ.get("BASS_DVE_PRELU", "5"))    # units preluing on DVE
POOL_RED = int(os.environ.get("BASS_POOL_RED", "22"))     # units whose tree-L1 runs on Pool


def _unit_flags():
    """Spread DVE-prelu and Pool-reduce units evenly over the 32 units."""
    units = [(s, fh) for s in range(NS) for fh in range(2)]
    n = len(units)
    dve_p = set()
    if DVE_PRELU > 0:
        step = n / DVE_PRELU
        dve_p = {units[min(n - 1, int(i * step))] for i in range(DVE_PRELU)}
    pool_r = set()
    if POOL_RED > 0:
        step = n / POOL_RED
        pool_r = {units[min(n - 1, int(i * step + 0.5))] for i in range(POOL_RED)}
    return dve_p, pool_r


def _build_nc():
    nc = bacc.Bacc("TRN2", target_bir_lowering=False, debug=False,
                   enable_asserts=True, num_devices=NCORES)

    # ---- per-core device I/O ----
    sT_d = nc.dram_tensor("stateT", [D, ROWS], BF16, kind="ExternalInput")
    pq_d = nc.dram_tensor("pq", [D, NS * 2 * NOBJ * NOBJ], BF16, kind="ExternalInput")
    wdeg_d = nc.dram_tensor("wdeg", [D, NS], BF16, kind="ExternalInput")
    ea_d = nc.dram_tensor("eafull", [2 * NOBJ, NS * 2 * NOBJ], BF16, kind="ExternalInput")
    w1a_d = nc.dram_tensor("w1aT", [D, F], BF16, kind="ExternalInput")
    w1b_d = nc.dram_tensor("w1bT", [D, F], BF16, kind="ExternalInput")
    w2k_d = nc.dram_tensor("w2T", [F, D], F32, kind="ExternalInput")
    fw1_d = nc.dram_tensor("fw1T", [2 * D, F], BF16, kind="ExternalInput")
    fw2_d = nc.dram_tensor("fw2T", [F, D], BF16, kind="ExternalInput")
    g1_d = nc.dram_tensor("g1", [F], F32, kind="ExternalInput")
    be1_d = nc.dram_tensor("be1", [F], F32, kind="ExternalInput")
    b2_d = nc.dram_tensor("b2", [D], F32, kind="ExternalInput")
    g2_d = nc.dram_tensor("g2", [F], F32, kind="ExternalInput")
    be2_d = nc.dram_tensor("be2", [F], F32, kind="ExternalInput")
    fb2_d = nc.dram_tensor("fb2", [D], F32, kind="ExternalInput")
    outT_d = nc.dram_tensor("outT", [D, ROWS], F32, kind="ExternalOutput")

    dve_prelu_units, pool_red_units = _unit_flags()

    from contextlib import ExitStack
    with tile.TileContext(nc) as tc, ExitStack() as ctx:
        consts = ctx.enter_context(tc.tile_pool(name="consts", bufs=1))
        uvp = ctx.enter_context(tc.tile_pool(name="uv", bufs=1))
        big = ctx.enter_context(tc.tile_pool(name="big", bufs=2))
        statp = ctx.enter_context(tc.tile_pool(name="stats", bufs=1))
        dram = ctx.enter_context(tc.tile_pool(name="dram", bufs=1, space="DRAM"))

        # ---------------- setup: load params + state ----------------
        sT = consts.tile([D, ROWS], BF16)
        nc.sync.dma_start(out=sT[:], in_=sT_d.ap())
        wdeg = consts.tile([D, NS], BF16)
        nc.sync.dma_start(out=wdeg[:], in_=wdeg_d.ap())
        # block-diag E pair-matrices for the cross-term
        eafull = consts.tile([2 * NOBJ, NS, 2 * NOBJ], BF16)
        nc.sync.dma_start(out=eafull[:],
                          in_=ea_d.ap().rearrange("p (s c) -> p s c", s=NS))
        pqt = consts.tile([D, NS, 2 * NOBJ * NOBJ], BF16)
        w1a = consts.tile([D, F], BF16)
        w1b = consts.tile([D, F], BF16)
        nc.sync.dma_start(out=w1a[:], in_=w1a_d.ap())
        nc.sync.dma_start(out=w1b[:], in_=w1b_d.ap())
        w2k = consts.tile([D, 2, D], F32)
        fw1 = consts.tile([D, 2, F], BF16)
        fw2 = consts.tile([D, 2, D], BF16)
        nc.sync.dma_start(out=w2k[:], in_=w2k_d.ap().rearrange("(k p) d -> p k d", p=D))
        nc.sync.dma_start(out=fw1[:], in_=fw1_d.ap().rearrange("(k p) f -> p k f", p=D))
        nc.sync.dma_start(out=fw2[:], in_=fw2_d.ap().rearrange("(k p) d -> p k d", p=D))

        def fvec(dh, nm):  # [256] dram vector -> [128, 2] feature-major sbuf
            t = consts.tile([D, 2], F32, tag=nm, name=nm)
            nc.sync.dma_start(out=t[:], in_=dh.ap().rearrange("(h p) -> p h", p=D))
            return t

        def dvec(dh, nm):  # [128] -> [128, 1]
            t = consts.tile([D, 1], F32, tag=nm, name=nm)
            nc.sync.dma_start(out=t[:], in_=dh.ap().rearrange("(h p) -> p h", p=D))
            return t

        g1c, be1c = fvec(g1_d, "g1c"), fvec(be1_d, "be1c")
        g2c, be2c = fvec(g2_d, "g2c"), fvec(be2_d, "be2c")
        b2c, fb2c = dvec(b2_d, "b2c"), dvec(fb2_d, "fb2c")
        b2x32 = consts.tile([D, 1], F32)
        nc.vector.tensor_scalar_mul(b2x32[:], b2c[:], float(NOBJ))
        ones1 = consts.tile([D, 1], BF16)
        nc.vector.memset(ones1[:], 1.0)
        # warm the ACT function tables (Prelu + Copy/Square/Sqrt set) while
        # the engines idle on input DMAs
        dum = consts.tile([1, 2], F32, name="dum")
        nc.vector.memset(dum[:], 1.0)
        dum2 = consts.tile([1, 2], F32, name="dum2")
        nc.scalar.activation(out=dum2[:], in_=dum[:], func=AF.Prelu, alpha=SLOPE)
        nc.scalar.activation(out=dum2[:], in_=dum[:], func=AF.Sqrt)

        # ------------- uvT matmuls: interleaved [u_b0; v_b0; u_b1; v_b1] ----
        # uvT[p, s, f]: p in [0,32) u rows of batch 2s (i=p), [32,64) v rows
        # of 2s, [64,96) u of 2s+1, [96,128) v of 2s+1.
        uvT = uvp.tile([D, NS, F], BF16)
        uvSq = uvp.tile([D, NS, F], BF16)
        sc1 = statp.tile([1, 2 * F], F32)
        psA_cm = tc.tile_pool(name="psA", bufs=2, space="PSUM")
        psA = psA_cm.__enter__()
        for s in range(NS):
            ps = psA.tile([D, F], F32, tag="uvps", bufs=2)
            rsl = slice(2 * s * NOBJ, (2 * s + 2) * NOBJ)   # 64 rows: b0|b1
            nc.tensor.matmul(ps[0:64, :], sT[:, rsl], w1a[:],
                             start=True, stop=True)
            nc.tensor.matmul(ps[64:128, :], sT[:, rsl], w1b[:],
                             start=True, stop=True)
            nc.scalar.activation(out=uvT[:, s, :], in_=ps[:], func=AF.Copy)
        with nc.allow_low_precision(reason="bf16 squares feed fp32 psum sums"):
            for h in range(2):
                nc.vector.tensor_mul(uvSq[:, 8 * h:8 * (h + 1), :],
                                     uvT[:, 8 * h:8 * (h + 1), :],
                                     uvT[:, 8 * h:8 * (h + 1), :])

        # ------------- BN1 stats: sum(H), sum(H^2) ----------------
        # sum(H)[f]  = sum_s sum_p wdeg[p,s]*uvT[p,s,f]   (deg for u rows,
        # cdeg for v rows, both batches)  -> [1, 256] psum accumulate
        shp = psA.tile([1, F], F32, tag="shp", bufs=1, name="shp")
        sqp = psA.tile([1, F], F32, tag="sqp", bufs=1, name="sqp")
        for s in range(NS):
            nc.tensor.matmul(shp[:], wdeg[:, s:s + 1], uvT[:, s, :],
                             start=(s == 0), stop=(s == NS - 1))
        for s in range(NS):
            nc.tensor.matmul(sqp[:], wdeg[:, s:s + 1], uvSq[:, s, :],
                             start=(s == 0), stop=(s == NS - 1))

        # cross term: sum e*u*v = sum_{b,i,f} u[(b,i),f] * (E_b v_b)[i,f].
        # ebv per slot lands on partitions [0:64) (i rows of b0|b1); multiply
        # by the matching u rows of uvT and column-sum via a ones-matmul.
        tm = uvp.tile([D, NS, F], BF16)
        crp = psA.tile([1, F], F32, tag="crp", bufs=1, name="crp")
        for s in range(NS):
            # ebtu[64+32h+j] = (E_b^T u_b)[j], landing on v_b's partitions
            ebtu = psA.tile([D, F], F32, tag="ebv", bufs=2)
            nc.tensor.matmul(ebtu[64:128, :], eafull[:, s, :],
                             uvT[0:64, s, :], start=True, stop=True)
            with nc.allow_low_precision(reason="cross-term partial"):
                nc.vector.tensor_mul(tm[64:128, s, :], uvT[64:128, s, :],
                                     ebtu[64:128, :])
        for s in range(NS):
            nc.tensor.matmul(crp[:], ones1[64:128, :], tm[64:128, s, :],
                             start=(s == 0), stop=(s == NS - 1))

        # pq is large (64KB/partition): issue late (the DMA-completion sem
        # serializes earlier compute behind it otherwise) and in 8 chunks
        # across two queues so early slots arrive first.
        pq_src = pq_d.ap().rearrange("p (s c) -> p s c", s=NS)
        for q in range(8):
            eng = nc.scalar if q % 2 == 0 else nc.gpsimd
            eng.dma_start(out=pqt[:, 2 * q:2 * (q + 1), :],
                          in_=pq_src[:, 2 * q:2 * (q + 1), :])

        # pack [sumH(256) | sumH2(256)] on one partition, roundtrip via DRAM
        # to feature-major [128, 4], AllReduce, then BN coeffs.
        nc.scalar.activation(out=sc1[:, 0:F], in_=shp[:], func=AF.Copy)
        nc.vector.scalar_tensor_tensor(out=sc1[:, F:2 * F], in0=crp[:], scalar=2.0,
                                       in1=sqp[:], op0=ALU.mult, op1=ALU.add)
        cc1_in = dram.tile([1, 2 * F], F32, tag="cc1i")
        nc.sync.dma_start(out=cc1_in[:], in_=sc1[:])
        cc1_out = dram.tile([1, 2 * F], F32, tag="cc1o")
        if NO_CC:
            nc.sync.dma_start(out=cc1_out[:], in_=cc1_in[:])
        else:
            nc.gpsimd.collective_compute(
                "AllReduce", ALU.add, replica_groups=[list(range(NCORES))],
                ins=[cc1_in[:].opt()], outs=[cc1_out[:].opt()])
        statg1 = statp.tile([D, 4], F32)
        nc.sync.dma_start(out=statg1[:],
                          in_=cc1_out[:].rearrange("x (g h p) -> (x p) (g h)",
                                                   p=D, g=2))

        epsc = statp.tile([D, 1], F32)
        nc.vector.memset(epsc[:], EPS)

        def bn_coeffs(statg, n_rows, gc, bec, pool, pre):
            def tl(nm):
                return pool.tile([D, 2], F32, tag=pre + nm, name=pre + nm)
            mean, msq, var = tl("mean"), tl("msq"), tl("var")
            sd, rstd, a, ma, z = tl("sd"), tl("rstd"), tl("a"), tl("ma"), tl("z")
            nc.scalar.activation(out=mean[:], in_=statg[:, 0:2], func=AF.Copy,
                                 scale=1.0 / n_rows)
            nc.scalar.activation(out=msq[:], in_=mean[:], func=AF.Square)
            nc.vector.scalar_tensor_tensor(
                out=var[:], in0=statg[:, 2:4], scalar=1.0 / n_rows,
                in1=msq[:], op0=ALU.mult, op1=ALU.subtract)
            nc.scalar.activation(out=sd[:], in_=var[:], func=AF.Sqrt, bias=epsc[:])
            nc.vector.reciprocal(out=rstd[:], in_=sd[:])
            nc.vector.tensor_mul(a[:], gc[:], rstd[:])
            nc.vector.tensor_mul(ma[:], mean[:], a[:])
            nc.vector.tensor_sub(z[:], bec[:], ma[:])
            return a, z

        a1, z1 = bn_coeffs(statg1, N1 / (NCORES if NO_CC else 1), g1c, be1c,
                           statp, "bn1_")

        # ------------- big phase: H on PE, Prelu, j-reduce ----------------
        # per unit (s, fh): H[128, 2048] = [b0 cols | b1 cols] in PSUM,
        #   m = Prelu(a1*H + z1)  (ACT, or DVE 2-instr for some units)
        #   msum[:, s*64:(s+1)*64] = sum_j m  (DVE halving tree or Pool)
        msumS = [statp.tile([D, ROWS], F32, tag=f"msum{h}", name=f"msum{h}")
                 for h in range(2)]
        psA_cm.__exit__(None, None, None)  # release psA banks for the big phase
        psB_cm = tc.tile_pool(name="psB", bufs=2, space="PSUM")
        psB = psB_cm.__enter__()
        CG2 = 2 * NOBJ * NOBJ  # 2048
        for s in range(NS):
            for fh in range(2):
                hps = psB.tile([D, CG2], F32, tag="hps", bufs=2)
                nc.tensor.matmul(hps[:], uvT[:, s, fh * D:(fh + 1) * D],
                                 pqt[:, s, :], start=True, stop=True)
                mt = big.tile([D, CG2], BF16, tag="mt", bufs=4, name=f"mt{s}{fh}")
                if (s, fh) in dve_prelu_units:
                    tf = big.tile([D, CG2], F32, tag="tf", bufs=3, name=f"tf{s}{fh}")
                    nc.vector.tensor_scalar(
                        out=tf[:], in0=hps[:], scalar1=a1[:, fh:fh + 1],
                        scalar2=z1[:, fh:fh + 1], op0=ALU.mult, op1=ALU.add)
                    with nc.allow_low_precision(reason="prelu out"):
                        nc.vector.scalar_tensor_tensor(
                            out=mt[:], in0=tf[:], scalar=SLOPE, in1=tf[:],
                            op0=ALU.mult, op1=ALU.max)
                else:
                    nc.scalar.activation(out=mt[:], in_=hps[:], func=AF.Prelu,
                                         scale=a1[:, fh:fh + 1],
                                         bias=z1[:, fh:fh + 1], alpha=SLOPE)
                osl = msumS[fh][:, s * 64:(s + 1) * 64]
                # packed-bf16 halving tree over j (2x DVE mode); level 1 may
                # run on Pool to balance engines
                cur = mt[:].rearrange("p (r j) -> p r j", j=NOBJ)
                w = NOBJ
                with nc.allow_low_precision(reason="bf16 j-tree"):
                    while w > 2:
                        w //= 2
                        nt = big.tile([D, 64 * w], BF16, tag=f"tr{w}",
                                      bufs=4, name=f"tr{s}{fh}{w}")
                        nv = nt[:].rearrange("p (r j) -> p r j", j=w)
                        eng = (nc.gpsimd if (w == NOBJ // 2 and
                                             (s, fh) in pool_red_units)
                               else nc.vector)
                        eng.tensor_add(nv, cur[:, :, 0:w], cur[:, :, w:2 * w])
                        cur = nv
                nc.vector.tensor_add(osl.rearrange("p (r x) -> p r x", x=1),
                                     cur[:, :, 0:1], cur[:, :, 1:2])

        # ------------- aggT = W2 @ msum + 32*b2 ; H2 = FW1 @ [sT; aggT] -----
        psB_cm.__exit__(None, None, None)
        psC = ctx.enter_context(tc.tile_pool(name="psC", bufs=2, space="PSUM"))
        aggT = statp.tile([D, ROWS], BF16, name="aggT")
        H2 = [statp.tile([D, ROWS], F32, tag=f"h2_{h}", name=f"h2_{h}")
              for h in range(2)]
        st2sum = statp.tile([D, 4], F32)   # col = fh*2 + nh : sum H2
        st2sq = statp.tile([D, 4], F32)    # col = fh*2 + nh : sum H2^2
        for nh in range(2):
            csl = slice(nh * 512, (nh + 1) * 512)
            ps = psC.tile([D, 512], F32, tag="aggp", bufs=2)
            nc.tensor.matmul(ps[:], w2k[:, 0, :], msumS[0][:, csl],
                             start=True, stop=False)
            nc.tensor.matmul(ps[:], w2k[:, 1, :], msumS[1][:, csl],
                             start=False, stop=True)
            nc.scalar.activation(out=aggT[:, csl], in_=ps[:],
                                 func=AF.Identity, bias=b2x32[:], scale=1.0)
            for fh in range(2):
                fsl = slice(fh * D, (fh + 1) * D)
                ps2 = psC.tile([D, 512], F32, tag="h2p", bufs=2)
                nc.tensor.matmul(ps2[:], fw1[:, 0, fsl], sT[:, csl],
                                 start=True, stop=False)
                nc.tensor.matmul(ps2[:], fw1[:, 1, fsl], aggT[:, csl],
                                 start=False, stop=True)
                c = fh * 2 + nh
                nc.scalar.activation(out=H2[fh][:, csl], in_=ps2[:],
                                     func=AF.Copy,
                                     accum_out=st2sum[:, c:c + 1])
                sq2 = statp.tile([D, 512], F32, tag="sq2")
                nc.vector.scalar_tensor_tensor(
                    out=sq2[:], in0=H2[fh][:, csl], scalar=1.0,
                    in1=H2[fh][:, csl], op0=ALU.mult, op1=ALU.mult,
                    accum_out=st2sq[:, c:c + 1])

        stat2 = statp.tile([D, 4], F32)
        nc.vector.reduce_sum(stat2[:, 0:2],
                             st2sum[:].rearrange("p (fh nh) -> p fh nh", nh=2),
                             axis=mybir.AxisListType.X)
        nc.vector.reduce_sum(stat2[:, 2:4],
                             st2sq[:].rearrange("p (fh nh) -> p fh nh", nh=2),
                             axis=mybir.AxisListType.X)
        cc2_in = dram.tile([D, 4], F32, tag="cc2i")
        cc2_out = dram.tile([D, 4], F32, tag="cc2o")
        nc.sync.dma_start(out=cc2_in[:], in_=stat2[:])
        if NO_CC:
            nc.sync.dma_start(out=cc2_out[:], in_=cc2_in[:])
        else:
            nc.gpsimd.collective_compute(
                "AllReduce", ALU.add, replica_groups=[list(range(NCORES))],
                ins=[cc2_in[:].opt()], outs=[cc2_out[:].opt()])
        statg2 = statp.tile([D, 4], F32)
        nc.sync.dma_start(out=statg2[:], in_=cc2_out[:])
        a2, z2 = bn_coeffs(statg2, N2 / (NCORES if NO_CC else 1), g2c, be2c,
                           statp, "bn2_")

        # ------------- m2 = Prelu(a2*H2+z2); outT = FW2 @ m2 + fb2 ---------
        m2 = [big.tile([D, ROWS], BF16, tag="m2", bufs=2, name=f"m2_{h}")
              for h in range(2)]
        for fh in range(2):
            nc.scalar.activation(out=m2[fh][:], in_=H2[fh][:], func=AF.Prelu,
                                 scale=a2[:, fh:fh + 1], bias=z2[:, fh:fh + 1],
                                 alpha=SLOPE)
        outT = statp.tile([D, ROWS], F32, name="outT")
        for nh in range(2):
            csl = slice(nh * 512, (nh + 1) * 512)
            ps = psC.tile([D, 512], F32, tag="outp", bufs=2)
            nc.tensor.matmul(ps[:], fw2[:, 0, :], m2[0][:, csl],
                             start=True, stop=False)
            nc.tensor.matmul(ps[:], fw2[:, 1, :], m2[1][:, csl],
                             start=False, stop=True)
            nc.scalar.activation(out=outT[:, csl], in_=ps[:],
                                 func=AF.Identity, bias=fb2c[:], scale=1.0)
        nc.sync.dma_start(out=outT_d.ap(), in_=outT[:])
    return nc


def _build_nc_staged():
    nc = _build_nc()
    nc.compile()
    return nc


_NC_CACHE = {}


def _get_nc():
    if "nc" not in _NC_CACHE:
        _NC_CACHE["nc"] = _build_nc_staged()
    return _NC_CACHE["nc"]


def _prep_in_maps(state, edges, msg_w1, msg_b1, msg_gamma, msg_beta, msg_w2,
                  msg_b2, fin_w1, fin_b1, fin_gamma, fin_beta, fin_w2, fin_b2,
                  **_unused):
    f32 = np.float32
    state = np.asarray(state, f32)
    edges = np.asarray(edges, f32)

    shared = {
        "w1aT": np.ascontiguousarray(np.asarray(msg_w1, f32)[:, :D].T).astype(NPBF),
        "w1bT": np.ascontiguousarray(np.asarray(msg_w1, f32)[:, D:].T).astype(NPBF),
        "w2T": np.ascontiguousarray(np.asarray(msg_w2, f32).T),
        "fw1T": np.ascontiguousarray(np.asarray(fin_w1, f32).T).astype(NPBF),
        "fw2T": np.ascontiguousarray(np.asarray(fin_w2, f32).T).astype(NPBF),
        "g1": np.ascontiguousarray(np.asarray(msg_gamma, f32)),
        "be1": np.ascontiguousarray(np.asarray(msg_beta, f32)),
        "b2": np.ascontiguousarray(np.asarray(msg_b2, f32)),
        "g2": np.ascontiguousarray(np.asarray(fin_gamma, f32)),
        "be2": np.ascontiguousarray(np.asarray(fin_beta, f32)),
        "fb2": np.ascontiguousarray(np.asarray(fin_b2, f32)),
    }
    idx = np.arange(NOBJ)
    in_maps = []
    for c in range(NCORES):
        sh = state[c * NB:(c + 1) * NB].reshape(ROWS, D)
        ed = edges[c * NB:(c + 1) * NB]          # [32, 1024]
        em = ed.reshape(NB, NOBJ, NOBJ)          # [b, i, j]
        deg = em.sum(axis=2)                     # [b, i]
        cdeg = em.sum(axis=1)                    # [b, j]
        # pq: [128, s, 2048]: cols = [b0(1024) | b1(1024)], partitions
        # [0:32) P(b0) on b0-cols, [32:64) P(b1) on b1-cols, [64:96) Q(b0),
        # [96:128) Q(b1); zero elsewhere, so one K=128 matmul computes both
        # batches.  uvT/wdeg use the matching [u_b0; u_b1; v_b0; v_b1] rows.
        pq = np.zeros((D, NS, 2, NOBJ * NOBJ), f32)
        wdeg = np.empty((D, NS), f32)
        ea = np.zeros((2 * NOBJ, NS, 2 * NOBJ), f32)
        for s in range(NS):
            for half in range(2):
                b = 2 * s + half
                P3 = pq[32 * half:32 * (half + 1), s, half].reshape(
                    NOBJ, NOBJ, NOBJ)
                P3[idx, idx, :] = em[b]          # P[k,k,:] = e[b,k,:]
                Q3 = pq[64 + 32 * half:64 + 32 * (half + 1), s, half].reshape(
                    NOBJ, NOBJ, NOBJ)
                Q3[idx, :, idx] = em[b].T        # Q[k,:,k] = e[b,:,k]
                wdeg[32 * half:32 * (half + 1), s] = deg[b]
                wdeg[64 + 32 * half:64 + 32 * (half + 1), s] = cdeg[b]
                ea[32 * half:32 * (half + 1), s,
                   32 * half:32 * (half + 1)] = em[b]
        in_maps.append({
            "stateT": np.ascontiguousarray(sh.T).astype(NPBF),
            "pq": np.ascontiguousarray(pq.reshape(D, -1)).astype(NPBF),
            "wdeg": np.ascontiguousarray(wdeg).astype(NPBF),
            "eafull": np.ascontiguousarray(ea.reshape(2 * NOBJ, -1)).astype(NPBF),
            **shared,
        })
    return in_maps


def kernel(**inputs):
    in_maps = _prep_in_maps(**inputs)
    nc = _get_nc()
    res = run_bass_kernel_spmd(nc, in_maps, core_ids=list(range(NCORES)))
    out = np.empty((B, NOBJ, D), np.float32)
    for c in range(NCORES):
        outT = res.results[c]["outT"]                       # [128, 1024]
        out[c * NB:(c + 1) * NB] = outT.T.reshape(NB, NOBJ, D)
    return out


if __name__ == "__main__":
    print("smoke-building nc...")
    _get_nc()
    print("built OK")
